# revision 16
# baseline (speedup 1.0000x reference)
"""AllSetTransformer hypergraph network on 8 TRN2 NeuronCores.

Sharding: nodes 12500/core, hyperedges 3750/core (padded-hedge global rows
8x3840=30720, which fits int16 for dma_gather).
  v2e blocks: per-core local U=[ex*Vx|ex] table (own node shard), chunked
    dma_gather with local int16 indices, one-hot M' matmuls reduce pin rows
    into PSUM partials over ALL hyperedge rows, bf16 ReduceScatter combines.
  e2v blocks: U table for hedge shard -> AllGather (full table), dma_gather
    with global padded-hedge idx, M' matmuls into local node-shard targets.
Target side: denom clamp+divide, +Q, LN0 (affine folded into W1/b1), MLP in
transposed layout (PE transposes), residual, LN1, relu.
Softmax max-subtraction skipped (shift-invariant, logits are O(1)).
"""
import sys
if '/opt/trn_rl_repo' not in sys.path:
    sys.path.insert(0, '/opt/trn_rl_repo')
import numpy as np
import ml_dtypes

bf = ml_dtypes.bfloat16

D, H, KD = 128, 4, 32
NCORES = 8
LN_EPS = 1e-5
P = 128


def _pad128(n):
    return ((n + 127) // 128) * 128


class Cfg:
    def __init__(self, NN, NH, E, chunk_tiles=8):
        self.NN, self.NH, self.E = NN, NH, E
        assert NN % NCORES == 0 and NH % NCORES == 0
        self.NS, self.HS = NN // NCORES, NH // NCORES
        self.NSP, self.HSP = _pad128(self.NS), _pad128(self.HS)
        self.NT_N, self.NT_H = self.NSP // P, self.HSP // P
        self.RT_H = NCORES * self.HSP      # padded global hedge rows
        assert self.RT_H <= 32767
        self.CH = chunk_tiles              # gather chunk size in 128-slot tiles
        self.TG = 16                       # target-side group in 128-row tiles


def _build_dir_structure(src_local, tgt_row, n_tb, cfg):
    """SPMD-shared tile structure + per-core gather idx / M' arrays."""
    per_core = []
    counts = np.zeros((NCORES, n_tb), np.int64)
    for c in range(NCORES):
        tb = tgt_row[c] // P
        order = np.argsort(tb, kind='stable')
        sl, tr = src_local[c][order], tgt_row[c][order]
        counts[c] = np.bincount(tb[order], minlength=n_tb)
        per_core.append((sl, tr))
    ntiles = np.maximum(1, (counts.max(0) + P - 1) // P).astype(np.int64)
    T = int(ntiles.sum())
    tb_of_tile = np.repeat(np.arange(n_tb), ntiles)
    tile_base = np.concatenate([[0], np.cumsum(ntiles)])
    S = T * P
    SPAD = ((S + cfg.CH * P - 1) // (cfg.CH * P)) * (cfg.CH * P)
    gidxs, mws = [], []
    for c in range(NCORES):
        sl, tr = per_core[c]
        gidx = np.zeros(SPAD, np.int16)
        mw = np.zeros((T, P, P), np.float32)
        starts = np.concatenate([[0], np.cumsum(counts[c])])
        for j in range(n_tb):
            cnt = int(counts[c][j])
            if cnt == 0:
                continue
            s0 = int(tile_base[j]) * P
            a = int(starts[j])
            slots = np.arange(s0, s0 + cnt)
            gidx[slots] = sl[a:a + cnt]
            tl = (tr[a:a + cnt] - j * P).astype(np.int64)
            np.add.at(mw, (slots // P, slots % P, tl), 1.0)
        packed = np.ascontiguousarray(gidx.reshape(-1, 16).T)   # [16, SPAD/16]
        gidxs.append(np.tile(packed, (8, 1)))                   # [128, SPAD/16]
        # pack M' chunk-contiguous: [nch, P, CH, P] so each chunk DMA is linear
        nch = (T + cfg.CH - 1) // cfg.CH
        mwp = np.zeros((nch, P, cfg.CH, P), np.float32)
        for g in range(nch):
            blk = mw[g * cfg.CH:(g + 1) * cfg.CH]
            mwp[g, :, :blk.shape[0], :] = blk.transpose(1, 0, 2)
        mws.append(mwp.astype(bf))
    return dict(ntiles=ntiles, tb_of_tile=tb_of_tile, T=T, S=S, SPAD=SPAD,
                gidx=gidxs, mw=mws)


def preprocess(inputs, cfg):
    node_idx = np.asarray(inputs['node_idx']).astype(np.int64)
    hedge_idx = np.asarray(inputs['hedge_idx']).astype(np.int64)
    owner = node_idx // cfg.NS
    hrows = (hedge_idx // cfg.HS) * cfg.HSP + (hedge_idx % cfg.HS)
    sv_src = [(node_idx[owner == c] % cfg.NS).astype(np.int16) for c in range(NCORES)]
    sv_tgt = [hrows[owner == c] for c in range(NCORES)]
    SV = _build_dir_structure(sv_src, sv_tgt, cfg.RT_H // P, cfg)
    se_src = [hrows[owner == c].astype(np.int16) for c in range(NCORES)]
    se_tgt = [node_idx[owner == c] % cfg.NS for c in range(NCORES)]
    SE = _build_dir_structure(se_src, se_tgt, cfg.NT_N, cfg)

    Kw = np.asarray(inputs['Kw'], np.float32)
    Vw = np.asarray(inputs['Vw'], np.float32)
    Qw = np.asarray(inputs['Qw'], np.float32)
    W1 = np.asarray(inputs['W1'], np.float32)
    b1 = np.asarray(inputs['b1'], np.float32)
    W2 = np.asarray(inputs['W2'], np.float32)
    b2 = np.asarray(inputs['b2'], np.float32)
    g0 = np.asarray(inputs['ln0_g'], np.float32)
    be0 = np.asarray(inputs['ln0_b'], np.float32)
    g1 = np.asarray(inputs['ln1_g'], np.float32)
    be1 = np.asarray(inputs['ln1_b'], np.float32)
    NB = Kw.shape[0]
    KVw = np.zeros((NB, D, 2 * D), np.float32)
    Qrep = np.zeros((NB, P, D), np.float32)
    W1p = np.zeros((NB, D, D), np.float32)
    b1p = np.zeros((NB, D), np.float32)
    for b in range(NB):
        KVw[b, :, :D] = Kw[b].transpose(1, 0, 2).reshape(D, D)
        KVw[b, :, D:] = Vw[b].transpose(1, 0, 2).reshape(D, D)
        Qrep[b] = np.tile(Qw[b].reshape(1, D), (P, 1))
        W1p[b] = g0[b][:, None] * W1[b]
        b1p[b] = b1[b] + be0[b] @ W1[b]
        assert np.allclose(g0[b], 1.0) and np.allclose(be0[b], 0.0), \
            "non-trivial ln0 affine on residual path not emitted"
        assert np.allclose(g1[b], 1.0) and np.allclose(be1[b], 0.0), \
            "non-trivial ln1 affine not emitted"
    x0 = np.asarray(inputs['x_0'], np.float32)
    wm = dict(KVw=KVw.astype(bf), Qrep=Qrep.astype(bf),
              Qflat=Qrep.astype(np.float32), W1p=W1p.astype(bf),
              b1p=b1p.astype(np.float32), W2p=W2.astype(bf),
              b2p=b2.astype(np.float32))
    in_maps = []
    for c in range(NCORES):
        xs = np.zeros((P, cfg.NSP), bf)
        xs[:, :cfg.NS] = x0[c * cfg.NS:(c + 1) * cfg.NS].T.astype(bf)
        m = dict(xT0=xs, gidxV=SV['gidx'][c], MwV=SV['mw'][c],
                 gidxE=SE['gidx'][c], MwE=SE['mw'][c])
        m.update(wm)
        in_maps.append(m)
    return SV, SE, in_maps


def build_graph(cfg, SV, SE, n_layers=2):
    from contextlib import ExitStack
    import concourse.bacc as bacc
    import concourse.tile as tile
    from concourse import mybir
    from concourse.masks import make_identity
    from concourse import library_config

    dt = mybir.dt
    Alu = mybir.AluOpType
    Act = mybir.ActivationFunctionType
    NB = 2 * n_layers

    nc = bacc.Bacc("TRN2", target_bir_lowering=False, debug=False,
                   num_devices=NCORES, num_swdge_queues=4)
    ein, eout = "ExternalInput", "ExternalOutput"
    xT0_d = nc.dram_tensor("xT0", [P, cfg.NSP], dt.bfloat16, kind=ein)
    gV_d = nc.dram_tensor("gidxV", [P, SV['SPAD'] // 16], dt.int16, kind=ein)
    mV_d = nc.dram_tensor("MwV", list(SV['mw'][0].shape), dt.bfloat16, kind=ein)
    gE_d = nc.dram_tensor("gidxE", [P, SE['SPAD'] // 16], dt.int16, kind=ein)
    mE_d = nc.dram_tensor("MwE", list(SE['mw'][0].shape), dt.bfloat16, kind=ein)
    KVw_d = nc.dram_tensor("KVw", [NB, D, 2 * D], dt.bfloat16, kind=ein)
    Qrep_d = nc.dram_tensor("Qrep", [NB, P, D], dt.bfloat16, kind=ein)
    Qflat_d = nc.dram_tensor("Qflat", [NB, P, D], dt.float32, kind=ein)
    W1_d = nc.dram_tensor("W1p", [NB, D, D], dt.bfloat16, kind=ein)
    b1_d = nc.dram_tensor("b1p", [NB, D], dt.float32, kind=ein)
    W2_d = nc.dram_tensor("W2p", [NB, D, D], dt.bfloat16, kind=ein)
    b2_d = nc.dram_tensor("b2p", [NB, D], dt.float32, kind=ein)
    x0o_d = nc.dram_tensor("x0o", [cfg.NS, D], dt.float32, kind=eout)
    x1o_d = nc.dram_tensor("x1o", [cfg.HS, D], dt.float32, kind=eout)

    es = ExitStack()
    with tile.TileContext(nc) as tc, es:
        nc.gpsimd.load_library(library_config.mlp)
        dram = es.enter_context(tc.tile_pool(name="dram", bufs=1, space="DRAM"))
        utable_n = dram.tile([cfg.NSP, 256], dt.bfloat16)
        ushard_h = dram.tile([cfg.HSP, 256], dt.bfloat16)
        utable_hs = [dram.tile([cfg.RT_H, 256], dt.bfloat16,
                              addr_space="Shared", name=f"utable_h{l}")
                     for l in range(n_layers)]
        partial = dram.tile([cfg.RT_H, 132], dt.bfloat16)
        rsout = dram.tile([cfg.HSP, 132], dt.bfloat16)

        const = es.enter_context(tc.tile_pool(name="const", bufs=1))
        ident = const.tile([P, P], dt.bfloat16)
        make_identity(nc, ident[:])
        KVw_s = const.tile([P, NB, 2 * D], dt.bfloat16)
        nc.sync.dma_start(out=KVw_s[:], in_=KVw_d.ap().rearrange("b d m -> d b m"))
        Qrep_s = const.tile([P, NB, D], dt.bfloat16)
        nc.sync.dma_start(out=Qrep_s[:], in_=Qrep_d.ap().rearrange("b p m -> p b m"))
        Qflat_s = const.tile([P, NB, D], dt.float32)
        nc.sync.dma_start(out=Qflat_s[:], in_=Qflat_d.ap().rearrange("b p m -> p b m"))
        W1_s = const.tile([P, NB, D], dt.bfloat16)
        nc.sync.dma_start(out=W1_s[:], in_=W1_d.ap().rearrange("b d m -> d b m"))
        W2_s = const.tile([P, NB, D], dt.bfloat16)
        nc.sync.dma_start(out=W2_s[:], in_=W2_d.ap().rearrange("b d m -> d b m"))
        b1_s = const.tile([P, NB], dt.float32)
        nc.sync.dma_start(out=b1_s[:], in_=b1_d.ap().rearrange("b d -> d b"))
        b2_s = const.tile([P, NB], dt.float32)
        nc.sync.dma_start(out=b2_s[:], in_=b2_d.ap().rearrange("b d -> d b"))
        gV_s = const.tile([P, SV['SPAD'] // 16], dt.int16)
        nc.sync.dma_start(out=gV_s[:], in_=gV_d.ap())
        gE_s = const.tile([P, SE['SPAD'] // 16], dt.int16)
        nc.sync.dma_start(out=gE_s[:], in_=gE_d.ap())

        state = es.enter_context(tc.tile_pool(name="state", bufs=1))
        xT_n = state.tile([P, cfg.NSP], dt.bfloat16)
        xT_h = state.tile([P, cfg.HSP], dt.bfloat16)
        nc.sync.dma_start(out=xT_n[:], in_=xT0_d.ap())

        psum = es.enter_context(tc.tile_pool(name="psum", bufs=2, space="PSUM"))
        work = es.enter_context(tc.tile_pool(name="work", bufs=2))
        big = es.enter_context(tc.tile_pool(name="big", bufs=1))

        def source_side(xT, ntiles, b, table):
            for i0 in range(0, ntiles, 8):
                ng = min(8, ntiles - i0)
                ub = work.tile([P, 8, 256], dt.bfloat16, tag="ub", name="ub")
                nc.vector.memset(ub[:, :, D + H:], 0)
                for k in range(ng):
                    i = i0 + k
                    kvps = psum.tile([P, 2 * D], dt.float32, tag="kvps", name="kvps")
                    nc.tensor.matmul(out=kvps[:], lhsT=xT[:, i * P:(i + 1) * P],
                                     rhs=KVw_s[:, b:b + 1, :].squeeze(1),
                                     start=True, stop=True)
                    lgt = work.tile([P, D], dt.float32, tag="lgt", name="lgt")
                    nc.vector.tensor_tensor(out=lgt[:], in0=kvps[:, 0:D],
                                            in1=Qrep_s[:, b:b + 1, :].squeeze(1),
                                            op=Alu.mult)
                    lg4 = work.tile([P, H], dt.float32, tag="lg4", name="lg4")
                    nc.vector.tensor_reduce(
                        out=lg4[:], in_=lgt[:].rearrange("p (h k) -> p h k", h=H),
                        axis=mybir.AxisListType.X, op=Alu.add)
                    ex4 = work.tile([P, H], dt.float32, tag="ex4", name="ex4")
                    nc.scalar.activation(out=ex4[:], in_=lg4[:], func=Act.Exp)
                    nc.vector.tensor_tensor(
                        out=ub[:, k, 0:D].rearrange("p (h k) -> p h k", h=H),
                        in0=kvps[:, D:2 * D].rearrange("p (h k) -> p h k", h=H),
                        in1=ex4[:].unsqueeze(2).to_broadcast([P, H, KD]),
                        op=Alu.mult)
                    nc.scalar.copy(out=ub[:, k, D:D + H], in_=ex4[:])
                nc.sync.dma_start(
                    out=table[i0 * P:(i0 + ng) * P, :].rearrange(
                        "(t p) d -> p t d", p=P),
                    in_=ub[:, 0:ng, :])

        import itertools
        _gq = itertools.count()

        def gather_reduce(table, gidx_s, mw_d, S, consume):
            T = S['T']
            tb_of = S['tb_of_tile']
            mpsums = {}
            for g in range((T + cfg.CH - 1) // cfg.CH):
                t0 = g * cfg.CH
                tch = min(cfg.CH, T - t0)
                nsl = tch * P
                gbuf = work.tile([P, cfg.CH, 256], dt.bfloat16, tag="gbuf",
                                 name="gbuf", bufs=4)
                nc.gpsimd.dma_gather(
                    gbuf[:, 0:tch, :], table[:, :],
                    gidx_s[:, t0 * 8:t0 * 8 + nsl // 16], nsl, nsl, 256,
                    queue_num=next(_gq) % 4)
                mbuf = work.tile([P, cfg.CH, P], dt.bfloat16, tag="mbuf",
                                 name="mbuf", bufs=4)
                nc.sync.dma_start(
                    out=mbuf[:, 0:tch, :],
                    in_=mw_d.ap()[g:g + 1, :, 0:tch, :].squeeze(0))
                for tl in range(tch):
                    t = t0 + tl
                    tb = int(tb_of[t])
                    first = (t == 0) or (tb_of[t - 1] != tb)
                    last = (t == T - 1) or (tb_of[t + 1] != tb)
                    if first:
                        mpsums[tb] = psum.tile([P, 132], dt.float32, tag="mpsum", name="mpsum")
                    nc.tensor.matmul(out=mpsums[tb][:],
                                     lhsT=mbuf[:, tl:tl + 1, :].squeeze(1),
                                     rhs=gbuf[:, tl:tl + 1, 0:132].squeeze(1),
                                     start=first, stop=last)
                    if last:
                        consume(tb, mpsums.pop(tb))

        def flush_rows(out_dram, buf, f0, n, rows_real):
            """DMA buf[:,0:n,:] (tile-major) to out_dram rows starting f0*128,
            clipping to rows_real."""
            r0 = f0 * P
            nfull = min(n, max(0, (rows_real - r0) // P))
            if nfull > 0:
                nc.sync.dma_start(
                    out=out_dram[r0:r0 + nfull * P, :].rearrange(
                        "(t p) d -> p t d", p=P),
                    in_=buf[:, 0:nfull, :])
            rem_r = r0 + nfull * P
            rem = rows_real - rem_r
            if 0 < rem < P and nfull < n:
                nc.sync.dma_start(out=out_dram[rem_r:rem_r + rem, :],
                                  in_=buf[0:rem, nfull:nfull + 1, :].squeeze(1))

        def target_side(agg_sb, Tt, b, xT_dest, out_dram, rows_real):
            for G0 in range(0, Tt, cfg.TG):
                TGn = min(cfg.TG, Tt - G0)
                NCOL = TGn * D
                asl = agg_sb[:, G0:G0 + TGn, :]
                dc = work.tile([P, cfg.TG, H], dt.float32, tag="dc", name="dc")
                nc.vector.tensor_scalar(out=dc[:, 0:TGn, :],
                                        in0=asl[:, :, D:D + H],
                                        scalar1=1e-20, scalar2=None, op0=Alu.max)
                rd = work.tile([P, cfg.TG, H], dt.float32, tag="rd", name="rd")
                nc.vector.reciprocal(out=rd[:, 0:TGn, :], in_=dc[:, 0:TGn, :])
                xq = big.tile([P, cfg.TG, D], dt.float32, tag="xq", name="xq")
                nc.vector.tensor_tensor(
                    out=xq[:, 0:TGn, :].rearrange("p t (h k) -> p t h k", h=H),
                    in0=asl[:, :, 0:D].rearrange("p t (h k) -> p t h k", h=H),
                    in1=rd[:, 0:TGn, :].unsqueeze(3).to_broadcast([P, TGn, H, KD]),
                    op=Alu.mult)
                nc.vector.tensor_tensor(
                    out=xq[:, 0:TGn, :], in0=xq[:, 0:TGn, :],
                    in1=Qflat_s[:, b:b + 1, :].to_broadcast([P, TGn, D]),
                    op=Alu.add)

                def ln_stats(src, tag):
                    ssum = work.tile([P, cfg.TG], dt.float32, tag=tag + "s")
                    nc.vector.tensor_reduce(out=ssum[:, 0:TGn], in_=src,
                                            axis=mybir.AxisListType.X, op=Alu.add)
                    sqt = big.tile([P, cfg.TG, D], dt.float32, tag="sqt", name="sqt")
                    nc.scalar.square(out=sqt[:, 0:TGn, :], in_=src)
                    s2 = work.tile([P, cfg.TG], dt.float32, tag=tag + "2")
                    nc.vector.tensor_reduce(out=s2[:, 0:TGn], in_=sqt[:, 0:TGn, :],
                                            axis=mybir.AxisListType.X, op=Alu.add)
                    mu = work.tile([P, cfg.TG], dt.float32, tag=tag + "m")
                    nc.vector.tensor_scalar(out=mu[:, 0:TGn], in0=ssum[:, 0:TGn],
                                            scalar1=1.0 / D, scalar2=None,
                                            op0=Alu.mult)
                    var = work.tile([P, cfg.TG], dt.float32, tag=tag + "v")
                    nc.vector.tensor_scalar(out=var[:, 0:TGn], in0=s2[:, 0:TGn],
                                            scalar1=1.0 / D, scalar2=None,
                                            op0=Alu.mult)
                    mu2 = work.tile([P, cfg.TG], dt.float32, tag=tag + "q")
                    nc.vector.tensor_tensor(out=mu2[:, 0:TGn], in0=mu[:, 0:TGn],
                                            in1=mu[:, 0:TGn], op=Alu.mult)
                    nc.vector.tensor_tensor(out=var[:, 0:TGn], in0=var[:, 0:TGn],
                                            in1=mu2[:, 0:TGn], op=Alu.subtract)
                    sd = work.tile([P, cfg.TG], dt.float32, tag=tag + "d")
                    nc.vector.tensor_scalar(out=sd[:, 0:TGn], in0=var[:, 0:TGn],
                                            scalar1=LN_EPS, scalar2=None,
                                            op0=Alu.add)
                    nc.scalar.sqrt(out=sd[:, 0:TGn], in_=sd[:, 0:TGn])
                    rsq = work.tile([P, cfg.TG], dt.float32, tag=tag + "r")
                    nc.vector.reciprocal(out=rsq[:, 0:TGn], in_=sd[:, 0:TGn])
                    nm = work.tile([P, cfg.TG], dt.float32, tag=tag + "n")
                    nc.vector.tensor_tensor(out=nm[:, 0:TGn], in0=mu[:, 0:TGn],
                                            in1=rsq[:, 0:TGn], op=Alu.mult)
                    nc.vector.tensor_scalar(out=nm[:, 0:TGn], in0=nm[:, 0:TGn],
                                            scalar1=-1.0, scalar2=None,
                                            op0=Alu.mult)
                    return rsq, nm

                rsq0, nm0 = ln_stats(xq[:, 0:TGn, :], "l0")
                tmp = big.tile([P, cfg.TG, D], dt.float32, tag="sqt", name="sqt")
                nc.vector.tensor_tensor(
                    out=tmp[:, 0:TGn, :], in0=xq[:, 0:TGn, :],
                    in1=rsq0[:, 0:TGn].unsqueeze(2).to_broadcast([P, TGn, D]),
                    op=Alu.mult)
                xh = big.tile([P, cfg.TG, D], dt.bfloat16, tag="xh", name="xh")
                nc.vector.tensor_tensor(
                    out=xh[:, 0:TGn, :], in0=tmp[:, 0:TGn, :],
                    in1=nm0[:, 0:TGn].unsqueeze(2).to_broadcast([P, TGn, D]),
                    op=Alu.add)
                xhT = big.tile([P, cfg.TG, D], dt.bfloat16, tag="xhT", name="xhT")
                for t in range(TGn):
                    tps = psum.tile([P, P], dt.bfloat16, tag="tps", name="tps")
                    nc.tensor.transpose(out=tps[:],
                                        in_=xh[:, t:t + 1, :].squeeze(1),
                                        identity=ident[:])
                    nc.scalar.copy(out=xhT[:, t, :], in_=tps[:])
                h1T = big.tile([P, cfg.TG, D], dt.bfloat16, tag="h1T", name="h1T")
                xhT_f = xhT[:].rearrange("p t d -> p (t d)")
                h1T_f = h1T[:].rearrange("p t d -> p (t d)")
                for c0 in range(0, NCOL, 512):
                    cw = min(512, NCOL - c0)
                    mlp1 = psum.tile([P, 512], dt.float32, tag="mlpps", name="mlpps")
                    nc.tensor.matmul(out=mlp1[:, 0:cw],
                                     lhsT=W1_s[:, b:b + 1, :].squeeze(1),
                                     rhs=xhT_f[:, c0:c0 + cw],
                                     start=True, stop=True)
                    nc.scalar.activation(out=h1T_f[:, c0:c0 + cw],
                                         in_=mlp1[:, 0:cw], func=Act.Relu,
                                         bias=b1_s[:, b:b + 1], scale=1.0)
                h2T = big.tile([P, cfg.TG, D], dt.bfloat16, tag="h2T", name="h2T")
                h2T_f = h2T[:].rearrange("p t d -> p (t d)")
                for c0 in range(0, NCOL, 512):
                    cw = min(512, NCOL - c0)
                    mlp2 = psum.tile([P, 512], dt.float32, tag="mlpps", name="mlpps")
                    nc.tensor.matmul(out=mlp2[:, 0:cw],
                                     lhsT=W2_s[:, b:b + 1, :].squeeze(1),
                                     rhs=h1T_f[:, c0:c0 + cw],
                                     start=True, stop=True)
                    nc.scalar.activation(out=h2T_f[:, c0:c0 + cw],
                                         in_=mlp2[:, 0:cw], func=Act.Identity,
                                         bias=b2_s[:, b:b + 1], scale=1.0)
                y = big.tile([P, cfg.TG, D], dt.float32, tag="xq", name="xq")
                for t in range(TGn):
                    tps2 = psum.tile([P, P], dt.bfloat16, tag="tps", name="tps")
                    nc.tensor.transpose(out=tps2[:],
                                        in_=h2T[:, t:t + 1, :].squeeze(1),
                                        identity=ident[:])
                    nc.vector.tensor_tensor(out=y[:, t, :],
                                            in0=xh[:, t:t + 1, :].squeeze(1),
                                            in1=tps2[:], op=Alu.add)
                rsq1, nm1 = ln_stats(y[:, 0:TGn, :], "l1")
                xob = None
                for t in range(TGn):
                    gt = G0 + t
                    if t % 8 == 0:
                        if xob is not None:
                            flush_rows(out_dram, xob, G0 + t - 8, 8, rows_real) \
                                if out_dram is not None else None
                        xob = work.tile([P, 8, D], dt.float32, tag="xob", name="xob")
                    nc.scalar.activation(out=xob[:, t % 8, :], in_=y[:, t:t + 1, :].squeeze(1),
                                         func=Act.Relu, scale=rsq1[:, t:t + 1],
                                         bias=nm1[:, t:t + 1])
                    xnb = work.tile([P, D], dt.bfloat16, tag="xnb", name="xnb")
                    nc.scalar.copy(out=xnb[:], in_=xob[:, t % 8, :])
                    tps3 = psum.tile([P, P], dt.bfloat16, tag="tps", name="tps")
                    nc.tensor.transpose(out=tps3[:], in_=xnb[:], identity=ident[:])
                    nc.vector.tensor_copy(xT_dest[:, gt * P:(gt + 1) * P], tps3[:])
                if out_dram is not None and xob is not None:
                    nlast = TGn - (TGn - 1) // 8 * 8
                    flush_rows(out_dram, xob, G0 + TGn - nlast, nlast, rows_real)

        rg = [list(range(NCORES))]
        for layer in range(n_layers):
            bv, bev = 2 * layer, 2 * layer + 1
            last = layer == n_layers - 1
            # ---- v2e ----
            source_side(xT_n, cfg.NT_N, bv, utable_n)
            pstate = {}

            def v2e_consume(tb, mp, pstate=pstate):
                if tb % 8 == 0:
                    pstate['pb'] = work.tile([P, 8, 132], dt.bfloat16, tag="pb", name="pb")
                nc.scalar.activation(out=pstate['pb'][:, tb % 8, :], in_=mp[:],
                                     func=Act.Copy)
                ntb = cfg.RT_H // P
                if tb % 8 == 7 or tb == ntb - 1:
                    n = tb % 8 + 1
                    nc.sync.dma_start(
                        out=partial[(tb - n + 1) * P:(tb + 1) * P, :].rearrange(
                            "(t p) d -> p t d", p=P),
                        in_=pstate['pb'][:, 0:n, :])
            gather_reduce(utable_n, gV_s, mV_d, SV, v2e_consume)
            nc.gpsimd.collective_compute(
                "ReduceScatter", Alu.add, replica_groups=rg,
                ins=[partial.opt()], outs=[rsout.opt()])
            agg_h = big.tile([P, cfg.NT_H, 132], dt.bfloat16, tag="aggh", name="aggh")
            nc.sync.dma_start(out=agg_h[:],
                              in_=rsout[:, :].rearrange("(t p) d -> p t d", p=P))
            target_side(agg_h, cfg.NT_H, bv, xT_h,
                        x1o_d.ap() if last else None, cfg.HS)
            # ---- e2v ----
            source_side(xT_h, cfg.NT_H, bev, ushard_h)
            utable_h = utable_hs[layer]
            nc.gpsimd.collective_compute(
                "AllGather", Alu.bypass, replica_groups=rg,
                ins=[ushard_h.opt()], outs=[utable_h.opt()])
            agg_n = big.tile([P, cfg.NT_N, 132], dt.bfloat16, tag="aggn", name="aggn")

            def e2v_consume(tb, mp, agg_n=agg_n):
                nc.scalar.activation(out=agg_n[:, tb, :], in_=mp[:], func=Act.Copy)
            gather_reduce(utable_h, gE_s, mE_d, SE, e2v_consume)
            target_side(agg_n, cfg.NT_N, bev, xT_n,
                        x0o_d.ap() if last else None, cfg.NS)

    nc.compile()
    return nc


def build(inputs, cfg=None):
    cfg = cfg or Cfg(100000, 30000, 400000)
    SV, SE, in_maps = preprocess(inputs, cfg)
    nc = build_graph(cfg, SV, SE)
    return nc, in_maps


def execute(nc, in_maps, trace=False):
    from concourse.bass_utils import run_bass_kernel_spmd
    res = run_bass_kernel_spmd(nc, in_maps, core_ids=list(range(NCORES)),
                               trace=trace)
    x0 = np.concatenate([np.asarray(res.results[c]['x0o']) for c in range(NCORES)], 0)
    x1 = np.concatenate([np.asarray(res.results[c]['x1o']) for c in range(NCORES)], 0)
    return (x0.astype(np.float32), x1.astype(np.float32)), res


def run(inputs, cfg=None, trace=False):
    nc, in_maps = build(inputs, cfg)
    return execute(nc, in_maps, trace=trace)


def kernel(**inputs):
    (x0, x1), _ = run(inputs)
    return (x0, x1)


# revision 19
# speedup vs baseline: 1.1174x; 1.1174x over previous
"""AllSetTransformer hypergraph network on 8 TRN2 NeuronCores.

Sharding: nodes 12500/core, hyperedges 3750/core (padded-hedge global rows
8x3840=30720, which fits int16 for dma_gather).
  v2e blocks: per-core local U=[ex*Vx|ex] table (own node shard), chunked
    dma_gather with local int16 indices, one-hot M' matmuls reduce pin rows
    into PSUM partials over ALL hyperedge rows, bf16 ReduceScatter combines.
  e2v blocks: U table for hedge shard -> AllGather (full table), dma_gather
    with global padded-hedge idx, M' matmuls into local node-shard targets.
Target side: denom clamp+divide, +Q, LN0 (affine folded into W1/b1), MLP in
transposed layout (PE transposes), residual, LN1, relu.
Softmax max-subtraction skipped (shift-invariant, logits are O(1)).
"""
import sys
if '/opt/trn_rl_repo' not in sys.path:
    sys.path.insert(0, '/opt/trn_rl_repo')
import numpy as np
import ml_dtypes

bf = ml_dtypes.bfloat16

D, H, KD = 128, 4, 32
NCORES = 8
LN_EPS = 1e-5
P = 128


def _pad128(n):
    return ((n + 127) // 128) * 128


class Cfg:
    def __init__(self, NN, NH, E, chunk_tiles=8):
        self.NN, self.NH, self.E = NN, NH, E
        assert NN % NCORES == 0 and NH % NCORES == 0
        self.NS, self.HS = NN // NCORES, NH // NCORES
        self.NSP, self.HSP = _pad128(self.NS), _pad128(self.HS)
        self.NT_N, self.NT_H = self.NSP // P, self.HSP // P
        self.RT_H = NCORES * self.HSP      # padded global hedge rows
        assert self.RT_H <= 32767
        self.CH = chunk_tiles              # gather chunk size in 128-slot tiles
        self.TG = 32                       # target-side group in 128-row tiles


def _build_dir_structure(src_local, tgt_row, n_tb, cfg):
    """SPMD-shared tile structure + per-core gather idx / M' arrays."""
    per_core = []
    counts = np.zeros((NCORES, n_tb), np.int64)
    for c in range(NCORES):
        tb = tgt_row[c] // P
        order = np.argsort(tb, kind='stable')
        sl, tr = src_local[c][order], tgt_row[c][order]
        counts[c] = np.bincount(tb[order], minlength=n_tb)
        per_core.append((sl, tr))
    ntiles = np.maximum(1, (counts.max(0) + P - 1) // P).astype(np.int64)
    T = int(ntiles.sum())
    tb_of_tile = np.repeat(np.arange(n_tb), ntiles)
    tile_base = np.concatenate([[0], np.cumsum(ntiles)])
    S = T * P
    SPAD = ((S + cfg.CH * P - 1) // (cfg.CH * P)) * (cfg.CH * P)
    gidxs, mws = [], []
    for c in range(NCORES):
        sl, tr = per_core[c]
        gidx = np.zeros(SPAD, np.int16)
        mw = np.zeros((T, P, P), np.float32)
        starts = np.concatenate([[0], np.cumsum(counts[c])])
        for j in range(n_tb):
            cnt = int(counts[c][j])
            if cnt == 0:
                continue
            s0 = int(tile_base[j]) * P
            a = int(starts[j])
            slots = np.arange(s0, s0 + cnt)
            gidx[slots] = sl[a:a + cnt]
            tl = (tr[a:a + cnt] - j * P).astype(np.int64)
            np.add.at(mw, (slots // P, slots % P, tl), 1.0)
        packed = np.ascontiguousarray(gidx.reshape(-1, 16).T)   # [16, SPAD/16]
        gidxs.append(np.tile(packed, (8, 1)))                   # [128, SPAD/16]
        # pack M' chunk-contiguous: [nch, P, CH, P] so each chunk DMA is linear
        nch = (T + cfg.CH - 1) // cfg.CH
        mwp = np.zeros((nch, P, cfg.CH, P), np.float32)
        for g in range(nch):
            blk = mw[g * cfg.CH:(g + 1) * cfg.CH]
            mwp[g, :, :blk.shape[0], :] = blk.transpose(1, 0, 2)
        mws.append(mwp.astype(bf))
    return dict(ntiles=ntiles, tb_of_tile=tb_of_tile, T=T, S=S, SPAD=SPAD,
                gidx=gidxs, mw=mws)


def preprocess(inputs, cfg):
    node_idx = np.asarray(inputs['node_idx']).astype(np.int64)
    hedge_idx = np.asarray(inputs['hedge_idx']).astype(np.int64)
    owner = node_idx // cfg.NS
    hrows = (hedge_idx // cfg.HS) * cfg.HSP + (hedge_idx % cfg.HS)
    sv_src = [(node_idx[owner == c] % cfg.NS).astype(np.int16) for c in range(NCORES)]
    sv_tgt = [hrows[owner == c] for c in range(NCORES)]
    SV = _build_dir_structure(sv_src, sv_tgt, cfg.RT_H // P, cfg)
    se_src = [hrows[owner == c].astype(np.int16) for c in range(NCORES)]
    se_tgt = [node_idx[owner == c] % cfg.NS for c in range(NCORES)]
    SE = _build_dir_structure(se_src, se_tgt, cfg.NT_N, cfg)

    Kw = np.asarray(inputs['Kw'], np.float32)
    Vw = np.asarray(inputs['Vw'], np.float32)
    Qw = np.asarray(inputs['Qw'], np.float32)
    W1 = np.asarray(inputs['W1'], np.float32)
    b1 = np.asarray(inputs['b1'], np.float32)
    W2 = np.asarray(inputs['W2'], np.float32)
    b2 = np.asarray(inputs['b2'], np.float32)
    g0 = np.asarray(inputs['ln0_g'], np.float32)
    be0 = np.asarray(inputs['ln0_b'], np.float32)
    g1 = np.asarray(inputs['ln1_g'], np.float32)
    be1 = np.asarray(inputs['ln1_b'], np.float32)
    NB = Kw.shape[0]
    KVw = np.zeros((NB, D, 2 * D), np.float32)
    Qrep = np.zeros((NB, P, D), np.float32)
    W1p = np.zeros((NB, D, D), np.float32)
    b1p = np.zeros((NB, D), np.float32)
    for b in range(NB):
        KVw[b, :, :D] = Kw[b].transpose(1, 0, 2).reshape(D, D)
        KVw[b, :, D:] = Vw[b].transpose(1, 0, 2).reshape(D, D)
        Qrep[b] = np.tile(Qw[b].reshape(1, D), (P, 1))
        W1p[b] = g0[b][:, None] * W1[b]
        b1p[b] = b1[b] + be0[b] @ W1[b]
        assert np.allclose(g0[b], 1.0) and np.allclose(be0[b], 0.0), \
            "non-trivial ln0 affine on residual path not emitted"
        assert np.allclose(g1[b], 1.0) and np.allclose(be1[b], 0.0), \
            "non-trivial ln1 affine not emitted"
    x0 = np.asarray(inputs['x_0'], np.float32)
    wm = dict(KVw=KVw.astype(bf), Qrep=Qrep.astype(bf),
              Qflat=Qrep.astype(np.float32), W1p=W1p.astype(bf),
              b1p=b1p.astype(np.float32), W2p=W2.astype(bf),
              b2p=b2.astype(np.float32))
    in_maps = []
    for c in range(NCORES):
        xs = np.zeros((P, cfg.NSP), bf)
        xs[:, :cfg.NS] = x0[c * cfg.NS:(c + 1) * cfg.NS].T.astype(bf)
        m = dict(xT0=xs, gidxV=SV['gidx'][c], MwV=SV['mw'][c],
                 gidxE=SE['gidx'][c], MwE=SE['mw'][c])
        m.update(wm)
        in_maps.append(m)
    return SV, SE, in_maps


def build_graph(cfg, SV, SE, n_layers=2):
    from contextlib import ExitStack
    import concourse.bacc as bacc
    import concourse.tile as tile
    from concourse import mybir
    from concourse.masks import make_identity
    from concourse import library_config

    dt = mybir.dt
    Alu = mybir.AluOpType
    Act = mybir.ActivationFunctionType
    NB = 2 * n_layers

    nc = bacc.Bacc("TRN2", target_bir_lowering=False, debug=False,
                   num_devices=NCORES, num_swdge_queues=4)
    ein, eout = "ExternalInput", "ExternalOutput"
    xT0_d = nc.dram_tensor("xT0", [P, cfg.NSP], dt.bfloat16, kind=ein)
    gV_d = nc.dram_tensor("gidxV", [P, SV['SPAD'] // 16], dt.int16, kind=ein)
    mV_d = nc.dram_tensor("MwV", list(SV['mw'][0].shape), dt.bfloat16, kind=ein)
    gE_d = nc.dram_tensor("gidxE", [P, SE['SPAD'] // 16], dt.int16, kind=ein)
    mE_d = nc.dram_tensor("MwE", list(SE['mw'][0].shape), dt.bfloat16, kind=ein)
    KVw_d = nc.dram_tensor("KVw", [NB, D, 2 * D], dt.bfloat16, kind=ein)
    Qrep_d = nc.dram_tensor("Qrep", [NB, P, D], dt.bfloat16, kind=ein)
    Qflat_d = nc.dram_tensor("Qflat", [NB, P, D], dt.float32, kind=ein)
    W1_d = nc.dram_tensor("W1p", [NB, D, D], dt.bfloat16, kind=ein)
    b1_d = nc.dram_tensor("b1p", [NB, D], dt.float32, kind=ein)
    W2_d = nc.dram_tensor("W2p", [NB, D, D], dt.bfloat16, kind=ein)
    b2_d = nc.dram_tensor("b2p", [NB, D], dt.float32, kind=ein)
    x0o_d = nc.dram_tensor("x0o", [cfg.NS, D], dt.float32, kind=eout)
    x1o_d = nc.dram_tensor("x1o", [cfg.HS, D], dt.float32, kind=eout)

    es = ExitStack()
    with tile.TileContext(nc) as tc, es:
        nc.gpsimd.load_library(library_config.mlp)
        dram = es.enter_context(tc.tile_pool(name="dram", bufs=1, space="DRAM"))
        utable_n = dram.tile([cfg.NSP, 256], dt.bfloat16)
        ushard_h = dram.tile([cfg.HSP, 256], dt.bfloat16)
        utable_hs = [dram.tile([cfg.RT_H, 256], dt.bfloat16,
                              addr_space="Shared", name=f"utable_h{l}")
                     for l in range(n_layers)]
        partial = dram.tile([cfg.RT_H, 132], dt.bfloat16)
        rsout = dram.tile([cfg.HSP, 132], dt.bfloat16)

        const = es.enter_context(tc.tile_pool(name="const", bufs=1))
        ident = const.tile([P, P], dt.bfloat16)
        make_identity(nc, ident[:])
        KVw_s = const.tile([P, NB, 2 * D], dt.bfloat16)
        nc.sync.dma_start(out=KVw_s[:], in_=KVw_d.ap().rearrange("b d m -> d b m"))
        Qrep_s = const.tile([P, NB, D], dt.bfloat16)
        nc.sync.dma_start(out=Qrep_s[:], in_=Qrep_d.ap().rearrange("b p m -> p b m"))
        Qflat_s = const.tile([P, NB, D], dt.float32)
        nc.sync.dma_start(out=Qflat_s[:], in_=Qflat_d.ap().rearrange("b p m -> p b m"))
        W1_s = const.tile([P, NB, D], dt.bfloat16)
        nc.sync.dma_start(out=W1_s[:], in_=W1_d.ap().rearrange("b d m -> d b m"))
        W2_s = const.tile([P, NB, D], dt.bfloat16)
        nc.sync.dma_start(out=W2_s[:], in_=W2_d.ap().rearrange("b d m -> d b m"))
        b1_s = const.tile([P, NB], dt.float32)
        nc.sync.dma_start(out=b1_s[:], in_=b1_d.ap().rearrange("b d -> d b"))
        b2_s = const.tile([P, NB], dt.float32)
        nc.sync.dma_start(out=b2_s[:], in_=b2_d.ap().rearrange("b d -> d b"))
        gV_s = const.tile([P, SV['SPAD'] // 16], dt.int16)
        nc.sync.dma_start(out=gV_s[:], in_=gV_d.ap())
        gE_s = const.tile([P, SE['SPAD'] // 16], dt.int16)
        nc.sync.dma_start(out=gE_s[:], in_=gE_d.ap())

        state = es.enter_context(tc.tile_pool(name="state", bufs=1))
        xT_n = state.tile([P, cfg.NSP], dt.bfloat16)
        xT_h = state.tile([P, cfg.HSP], dt.bfloat16)
        nc.sync.dma_start(out=xT_n[:], in_=xT0_d.ap())

        psum = es.enter_context(tc.tile_pool(name="psum", bufs=2, space="PSUM"))
        work = es.enter_context(tc.tile_pool(name="work", bufs=2))
        big = es.enter_context(tc.tile_pool(name="big", bufs=1))

        def source_side(xT, ntiles, b, table):
            for i0 in range(0, ntiles, 8):
                ng = min(8, ntiles - i0)
                ub = work.tile([P, 8, 256], dt.bfloat16, tag="ub", name="ub", bufs=3)
                nc.vector.memset(ub[:, :, D + H:], 0)
                for k in range(ng):
                    i = i0 + k
                    kvps = psum.tile([P, 2 * D], dt.float32, tag="kvps", name="kvps")
                    nc.tensor.matmul(out=kvps[:], lhsT=xT[:, i * P:(i + 1) * P],
                                     rhs=KVw_s[:, b:b + 1, :].squeeze(1),
                                     start=True, stop=True)
                    lgt = work.tile([P, D], dt.float32, tag="lgt", name="lgt", bufs=4)
                    nc.vector.tensor_tensor(out=lgt[:], in0=kvps[:, 0:D],
                                            in1=Qrep_s[:, b:b + 1, :].squeeze(1),
                                            op=Alu.mult)
                    lg4 = work.tile([P, H], dt.float32, tag="lg4", name="lg4", bufs=4)
                    nc.vector.tensor_reduce(
                        out=lg4[:], in_=lgt[:].rearrange("p (h k) -> p h k", h=H),
                        axis=mybir.AxisListType.X, op=Alu.add)
                    ex4 = work.tile([P, H], dt.float32, tag="ex4", name="ex4", bufs=4)
                    nc.scalar.activation(out=ex4[:], in_=lg4[:], func=Act.Exp)
                    nc.vector.tensor_tensor(
                        out=ub[:, k, 0:D].rearrange("p (h k) -> p h k", h=H),
                        in0=kvps[:, D:2 * D].rearrange("p (h k) -> p h k", h=H),
                        in1=ex4[:].unsqueeze(2).to_broadcast([P, H, KD]),
                        op=Alu.mult)
                    nc.scalar.copy(out=ub[:, k, D:D + H], in_=ex4[:])
                nc.sync.dma_start(
                    out=table[i0 * P:(i0 + ng) * P, :].rearrange(
                        "(t p) d -> p t d", p=P),
                    in_=ub[:, 0:ng, :])

        import itertools
        _gq = itertools.count()

        def gather_reduce(table, gidx_s, mw_d, S, consume):
            T = S['T']
            tb_of = S['tb_of_tile']
            mpsums = {}
            for g in range((T + cfg.CH - 1) // cfg.CH):
                t0 = g * cfg.CH
                tch = min(cfg.CH, T - t0)
                nsl = tch * P
                gbuf = work.tile([P, cfg.CH, 256], dt.bfloat16, tag="gbuf",
                                 name="gbuf", bufs=4)
                nc.gpsimd.dma_gather(
                    gbuf[:, 0:tch, :], table[:, :],
                    gidx_s[:, t0 * 8:t0 * 8 + nsl // 16], nsl, nsl, 256,
                    queue_num=next(_gq) % 4)
                mbuf = work.tile([P, cfg.CH, P], dt.bfloat16, tag="mbuf",
                                 name="mbuf", bufs=4)
                nc.sync.dma_start(
                    out=mbuf[:, 0:tch, :],
                    in_=mw_d.ap()[g:g + 1, :, 0:tch, :].squeeze(0))
                for tl in range(tch):
                    t = t0 + tl
                    tb = int(tb_of[t])
                    first = (t == 0) or (tb_of[t - 1] != tb)
                    last = (t == T - 1) or (tb_of[t + 1] != tb)
                    if first:
                        mpsums[tb] = psum.tile([P, 132], dt.float32, tag="mpsum", name="mpsum")
                    nc.tensor.matmul(out=mpsums[tb][:],
                                     lhsT=mbuf[:, tl:tl + 1, :].squeeze(1),
                                     rhs=gbuf[:, tl:tl + 1, 0:132].squeeze(1),
                                     start=first, stop=last)
                    if last:
                        consume(tb, mpsums.pop(tb))

        def flush_rows(out_dram, buf, f0, n, rows_real):
            """DMA buf[:,0:n,:] (tile-major) to out_dram rows starting f0*128,
            clipping to rows_real."""
            r0 = f0 * P
            nfull = min(n, max(0, (rows_real - r0) // P))
            if nfull > 0:
                nc.sync.dma_start(
                    out=out_dram[r0:r0 + nfull * P, :].rearrange(
                        "(t p) d -> p t d", p=P),
                    in_=buf[:, 0:nfull, :])
            rem_r = r0 + nfull * P
            rem = rows_real - rem_r
            if 0 < rem < P and nfull < n:
                nc.sync.dma_start(out=out_dram[rem_r:rem_r + rem, :],
                                  in_=buf[0:rem, nfull:nfull + 1, :].squeeze(1))

        def target_side(agg_sb, Tt, b, xT_dest, out_dram, rows_real,
                        tile_off=0):
            for G0 in range(0, Tt, cfg.TG):
                TGn = min(cfg.TG, Tt - G0)
                NCOL = TGn * D
                asl = agg_sb[:, G0:G0 + TGn, :]
                dc = work.tile([P, cfg.TG, H], dt.float32, tag="dc", name="dc")
                nc.vector.tensor_scalar(out=dc[:, 0:TGn, :],
                                        in0=asl[:, :, D:D + H],
                                        scalar1=1e-20, scalar2=None, op0=Alu.max)
                rd = work.tile([P, cfg.TG, H], dt.float32, tag="rd", name="rd")
                nc.vector.reciprocal(out=rd[:, 0:TGn, :], in_=dc[:, 0:TGn, :])
                xq = big.tile([P, cfg.TG, D], dt.float32, tag="xq", name="xq")
                nc.vector.tensor_tensor(
                    out=xq[:, 0:TGn, :].rearrange("p t (h k) -> p t h k", h=H),
                    in0=asl[:, :, 0:D].rearrange("p t (h k) -> p t h k", h=H),
                    in1=rd[:, 0:TGn, :].unsqueeze(3).to_broadcast([P, TGn, H, KD]),
                    op=Alu.mult)
                l0s = work.tile([P, cfg.TG], dt.float32, tag="l0s", name="l0s")
                for t in range(TGn):
                    nc.vector.scalar_tensor_tensor(
                        out=xq[:, t, :], in0=xq[:, t, :], scalar=1.0,
                        in1=Qflat_s[:, b, :], op0=Alu.mult, op1=Alu.add,
                        accum_out=l0s[:, t:t + 1])

                def ln_stats(src_tiles, ssum, tag):
                    # per-tile Square with accum_out gives row sum-of-squares
                    s2 = work.tile([P, cfg.TG], dt.float32, tag=tag + "2")
                    for t in range(TGn):
                        sqscr = work.tile([P, D], dt.float32, tag="sqscr",
                                          name="sqscr")
                        nc.scalar.activation(out=sqscr[:], in_=src_tiles(t),
                                             func=Act.Square,
                                             accum_out=s2[:, t:t + 1])
                    mu = work.tile([P, cfg.TG], dt.float32, tag=tag + "m")
                    nc.vector.tensor_scalar(out=mu[:, 0:TGn], in0=ssum[:, 0:TGn],
                                            scalar1=1.0 / D, scalar2=None,
                                            op0=Alu.mult)
                    var = work.tile([P, cfg.TG], dt.float32, tag=tag + "v")
                    nc.vector.tensor_scalar(out=var[:, 0:TGn], in0=s2[:, 0:TGn],
                                            scalar1=1.0 / D, scalar2=None,
                                            op0=Alu.mult)
                    mu2 = work.tile([P, cfg.TG], dt.float32, tag=tag + "q")
                    nc.vector.tensor_tensor(out=mu2[:, 0:TGn], in0=mu[:, 0:TGn],
                                            in1=mu[:, 0:TGn], op=Alu.mult)
                    nc.vector.tensor_tensor(out=var[:, 0:TGn], in0=var[:, 0:TGn],
                                            in1=mu2[:, 0:TGn], op=Alu.subtract)
                    sd = work.tile([P, cfg.TG], dt.float32, tag=tag + "d")
                    nc.vector.tensor_scalar(out=sd[:, 0:TGn], in0=var[:, 0:TGn],
                                            scalar1=LN_EPS, scalar2=None,
                                            op0=Alu.add)
                    nc.scalar.sqrt(out=sd[:, 0:TGn], in_=sd[:, 0:TGn])
                    rsq = work.tile([P, cfg.TG], dt.float32, tag=tag + "r")
                    nc.vector.reciprocal(out=rsq[:, 0:TGn], in_=sd[:, 0:TGn])
                    nm = work.tile([P, cfg.TG], dt.float32, tag=tag + "n")
                    nc.vector.tensor_tensor(out=nm[:, 0:TGn], in0=mu[:, 0:TGn],
                                            in1=rsq[:, 0:TGn], op=Alu.mult)
                    nc.vector.tensor_scalar(out=nm[:, 0:TGn], in0=nm[:, 0:TGn],
                                            scalar1=-1.0, scalar2=None,
                                            op0=Alu.mult)
                    return rsq, nm

                rsq0, nm0 = ln_stats(lambda t: xq[:, t, :], l0s, "l0")
                xh = big.tile([P, cfg.TG, D], dt.bfloat16, tag="xh", name="xh")
                for t in range(TGn):
                    nc.vector.scalar_tensor_tensor(
                        out=xh[:, t, :], in0=xq[:, t, :],
                        scalar=rsq0[:, t:t + 1],
                        in1=nm0[:, t:t + 1].to_broadcast([P, D]),
                        op0=Alu.mult, op1=Alu.add)
                xhT = big.tile([P, cfg.TG, D], dt.bfloat16, tag="xhT", name="xhT")
                for t in range(TGn):
                    tps = psum.tile([P, P], dt.bfloat16, tag="tps", name="tps")
                    nc.tensor.transpose(out=tps[:],
                                        in_=xh[:, t:t + 1, :].squeeze(1),
                                        identity=ident[:])
                    nc.scalar.copy(out=xhT[:, t, :], in_=tps[:])
                h1T = big.tile([P, cfg.TG, D], dt.bfloat16, tag="h1T", name="h1T")
                xhT_f = xhT[:].rearrange("p t d -> p (t d)")
                h1T_f = h1T[:].rearrange("p t d -> p (t d)")
                for c0 in range(0, NCOL, 512):
                    cw = min(512, NCOL - c0)
                    mlp1 = psum.tile([P, 512], dt.float32, tag="mlpps", name="mlpps")
                    nc.tensor.matmul(out=mlp1[:, 0:cw],
                                     lhsT=W1_s[:, b:b + 1, :].squeeze(1),
                                     rhs=xhT_f[:, c0:c0 + cw],
                                     start=True, stop=True)
                    nc.scalar.activation(out=h1T_f[:, c0:c0 + cw],
                                         in_=mlp1[:, 0:cw], func=Act.Relu,
                                         bias=b1_s[:, b:b + 1], scale=1.0)
                h2T = big.tile([P, cfg.TG, D], dt.bfloat16, tag="h2T", name="h2T")
                h2T_f = h2T[:].rearrange("p t d -> p (t d)")
                for c0 in range(0, NCOL, 512):
                    cw = min(512, NCOL - c0)
                    mlp2 = psum.tile([P, 512], dt.float32, tag="mlpps", name="mlpps")
                    nc.tensor.matmul(out=mlp2[:, 0:cw],
                                     lhsT=W2_s[:, b:b + 1, :].squeeze(1),
                                     rhs=h1T_f[:, c0:c0 + cw],
                                     start=True, stop=True)
                    nc.scalar.activation(out=h2T_f[:, c0:c0 + cw],
                                         in_=mlp2[:, 0:cw], func=Act.Identity,
                                         bias=b2_s[:, b:b + 1], scale=1.0)
                y = big.tile([P, cfg.TG, D], dt.float32, tag="xq", name="xq")
                l1s = work.tile([P, cfg.TG], dt.float32, tag="l1s", name="l1s")
                for t in range(TGn):
                    tps2 = psum.tile([P, P], dt.bfloat16, tag="tps", name="tps")
                    nc.tensor.transpose(out=tps2[:],
                                        in_=h2T[:, t:t + 1, :].squeeze(1),
                                        identity=ident[:])
                    nc.vector.scalar_tensor_tensor(
                        out=y[:, t, :], in0=xh[:, t:t + 1, :].squeeze(1),
                        scalar=1.0, in1=tps2[:], op0=Alu.mult, op1=Alu.add,
                        accum_out=l1s[:, t:t + 1])
                rsq1, nm1 = ln_stats(lambda t: y[:, t, :], l1s, "l1")
                xob = None
                for t in range(TGn):
                    gt = tile_off + G0 + t
                    if t % 8 == 0:
                        if xob is not None:
                            flush_rows(out_dram, xob, tile_off + G0 + t - 8, 8,
                                       rows_real) \
                                if out_dram is not None else None
                        xob = work.tile([P, 8, D], dt.float32, tag="xob", name="xob")
                    nc.scalar.activation(out=xob[:, t % 8, :], in_=y[:, t:t + 1, :].squeeze(1),
                                         func=Act.Relu, scale=rsq1[:, t:t + 1],
                                         bias=nm1[:, t:t + 1])
                    xnb = work.tile([P, D], dt.bfloat16, tag="xnb", name="xnb")
                    nc.scalar.copy(out=xnb[:], in_=xob[:, t % 8, :])
                    tps3 = psum.tile([P, P], dt.bfloat16, tag="tps", name="tps")
                    nc.tensor.transpose(out=tps3[:], in_=xnb[:], identity=ident[:])
                    nc.vector.tensor_copy(xT_dest[:, gt * P:(gt + 1) * P], tps3[:])
                if out_dram is not None and xob is not None:
                    nlast = TGn - (TGn - 1) // 8 * 8
                    flush_rows(out_dram, xob, tile_off + G0 + TGn - nlast, nlast,
                               rows_real)

        rg = [list(range(NCORES))]
        for layer in range(n_layers):
            bv, bev = 2 * layer, 2 * layer + 1
            last = layer == n_layers - 1
            # ---- v2e ----
            source_side(xT_n, cfg.NT_N, bv, utable_n)
            pstate = {}

            def v2e_consume(tb, mp, pstate=pstate):
                if tb % 8 == 0:
                    pstate['pb'] = work.tile([P, 8, 132], dt.bfloat16, tag="pb", name="pb")
                nc.scalar.activation(out=pstate['pb'][:, tb % 8, :], in_=mp[:],
                                     func=Act.Copy)
                ntb = cfg.RT_H // P
                if tb % 8 == 7 or tb == ntb - 1:
                    n = tb % 8 + 1
                    nc.sync.dma_start(
                        out=partial[(tb - n + 1) * P:(tb + 1) * P, :].rearrange(
                            "(t p) d -> p t d", p=P),
                        in_=pstate['pb'][:, 0:n, :])
            gather_reduce(utable_n, gV_s, mV_d, SV, v2e_consume)
            nc.gpsimd.collective_compute(
                "ReduceScatter", Alu.add, replica_groups=rg,
                ins=[partial.opt()], outs=[rsout.opt()])
            agg_h = big.tile([P, cfg.NT_H, 132], dt.bfloat16, tag="aggh", name="aggh")
            nc.sync.dma_start(out=agg_h[:],
                              in_=rsout[:, :].rearrange("(t p) d -> p t d", p=P))
            target_side(agg_h, cfg.NT_H, bv, xT_h,
                        x1o_d.ap() if last else None, cfg.HS)
            # ---- e2v ----
            source_side(xT_h, cfg.NT_H, bev, ushard_h)
            utable_h = utable_hs[layer]
            nc.gpsimd.collective_compute(
                "AllGather", Alu.bypass, replica_groups=rg,
                ins=[ushard_h.opt()], outs=[utable_h.opt()])
            ngr = (cfg.NT_N + cfg.TG - 1) // cfg.TG
            agg_n_g = [big.tile([P, min(cfg.TG, cfg.NT_N - gi * cfg.TG), 132],
                               dt.bfloat16, tag=f"aggn{gi}", name=f"aggn{gi}")
                       for gi in range(ngr)]
            e2v_out = x0o_d.ap() if last else None

            def e2v_consume(tb, mp, agg_n_g=agg_n_g, bev=bev, e2v_out=e2v_out):
                gi = tb // cfg.TG
                nc.scalar.activation(out=agg_n_g[gi][:, tb % cfg.TG, :],
                                     in_=mp[:], func=Act.Copy)
                if tb == min((gi + 1) * cfg.TG, cfg.NT_N) - 1:
                    target_side(agg_n_g[gi], agg_n_g[gi].shape[1], bev, xT_n,
                                e2v_out, cfg.NS, tile_off=gi * cfg.TG)
            gather_reduce(utable_h, gE_s, mE_d, SE, e2v_consume)

    nc.compile()
    return nc


def build(inputs, cfg=None):
    cfg = cfg or Cfg(100000, 30000, 400000)
    SV, SE, in_maps = preprocess(inputs, cfg)
    nc = build_graph(cfg, SV, SE)
    return nc, in_maps


def execute(nc, in_maps, trace=False):
    from concourse.bass_utils import run_bass_kernel_spmd
    res = run_bass_kernel_spmd(nc, in_maps, core_ids=list(range(NCORES)),
                               trace=trace)
    x0 = np.concatenate([np.asarray(res.results[c]['x0o']) for c in range(NCORES)], 0)
    x1 = np.concatenate([np.asarray(res.results[c]['x1o']) for c in range(NCORES)], 0)
    return (x0.astype(np.float32), x1.astype(np.float32)), res


def run(inputs, cfg=None, trace=False):
    nc, in_maps = build(inputs, cfg)
    return execute(nc, in_maps, trace=trace)


def kernel(**inputs):
    (x0, x1), _ = run(inputs)
    return (x0, x1)


# revision 21
# speedup vs baseline: 1.1184x; 1.0009x over previous
"""AllSetTransformer hypergraph network on 8 TRN2 NeuronCores.

Sharding: nodes 12500/core, hyperedges 3750/core (padded-hedge global rows
8x3840=30720, which fits int16 for dma_gather).
  v2e blocks: per-core local U=[ex*Vx|ex] table (own node shard), chunked
    dma_gather with local int16 indices, one-hot M' matmuls reduce pin rows
    into PSUM partials over ALL hyperedge rows, bf16 ReduceScatter combines.
  e2v blocks: U table for hedge shard -> AllGather (full table), dma_gather
    with global padded-hedge idx, M' matmuls into local node-shard targets.
Target side: denom clamp+divide, +Q, LN0 (affine folded into W1/b1), MLP in
transposed layout (PE transposes), residual, LN1, relu.
Softmax max-subtraction skipped (shift-invariant, logits are O(1)).
"""
import sys
if '/opt/trn_rl_repo' not in sys.path:
    sys.path.insert(0, '/opt/trn_rl_repo')
import numpy as np
import ml_dtypes

bf = ml_dtypes.bfloat16

D, H, KD = 128, 4, 32
NCORES = 8
LN_EPS = 1e-5
P = 128


def _pad128(n):
    return ((n + 127) // 128) * 128


class Cfg:
    def __init__(self, NN, NH, E, chunk_tiles=8):
        self.NN, self.NH, self.E = NN, NH, E
        assert NN % NCORES == 0 and NH % NCORES == 0
        self.NS, self.HS = NN // NCORES, NH // NCORES
        self.NSP, self.HSP = _pad128(self.NS), _pad128(self.HS)
        self.NT_N, self.NT_H = self.NSP // P, self.HSP // P
        self.RT_H = NCORES * self.HSP      # padded global hedge rows
        assert self.RT_H <= 32767
        self.CH = chunk_tiles              # gather chunk size in 128-slot tiles
        self.TG = 16                       # target-side group in 128-row tiles


def _build_dir_structure(src_local, tgt_row, n_tb, cfg):
    """SPMD-shared tile structure + per-core gather idx / M' arrays."""
    per_core = []
    counts = np.zeros((NCORES, n_tb), np.int64)
    for c in range(NCORES):
        tb = tgt_row[c] // P
        order = np.argsort(tb, kind='stable')
        sl, tr = src_local[c][order], tgt_row[c][order]
        counts[c] = np.bincount(tb[order], minlength=n_tb)
        per_core.append((sl, tr))
    ntiles = np.maximum(1, (counts.max(0) + P - 1) // P).astype(np.int64)
    T = int(ntiles.sum())
    tb_of_tile = np.repeat(np.arange(n_tb), ntiles)
    tile_base = np.concatenate([[0], np.cumsum(ntiles)])
    S = T * P
    SPAD = ((S + cfg.CH * P - 1) // (cfg.CH * P)) * (cfg.CH * P)
    gidxs, mws = [], []
    for c in range(NCORES):
        sl, tr = per_core[c]
        gidx = np.zeros(SPAD, np.int16)
        mw = np.zeros((T, P, P), np.float32)
        starts = np.concatenate([[0], np.cumsum(counts[c])])
        for j in range(n_tb):
            cnt = int(counts[c][j])
            if cnt == 0:
                continue
            s0 = int(tile_base[j]) * P
            a = int(starts[j])
            slots = np.arange(s0, s0 + cnt)
            gidx[slots] = sl[a:a + cnt]
            tl = (tr[a:a + cnt] - j * P).astype(np.int64)
            np.add.at(mw, (slots // P, slots % P, tl), 1.0)
        packed = np.ascontiguousarray(gidx.reshape(-1, 16).T)   # [16, SPAD/16]
        gidxs.append(np.tile(packed, (8, 1)))                   # [128, SPAD/16]
        # pack M' chunk-contiguous: [nch, P, CH, P] so each chunk DMA is linear
        nch = (T + cfg.CH - 1) // cfg.CH
        mwp = np.zeros((nch, P, cfg.CH, P), np.float32)
        for g in range(nch):
            blk = mw[g * cfg.CH:(g + 1) * cfg.CH]
            mwp[g, :, :blk.shape[0], :] = blk.transpose(1, 0, 2)
        mws.append(mwp.astype(bf))
    return dict(ntiles=ntiles, tb_of_tile=tb_of_tile, T=T, S=S, SPAD=SPAD,
                gidx=gidxs, mw=mws)


def preprocess(inputs, cfg):
    node_idx = np.asarray(inputs['node_idx']).astype(np.int64)
    hedge_idx = np.asarray(inputs['hedge_idx']).astype(np.int64)
    owner = node_idx // cfg.NS
    hrows = (hedge_idx // cfg.HS) * cfg.HSP + (hedge_idx % cfg.HS)
    sv_src = [(node_idx[owner == c] % cfg.NS).astype(np.int16) for c in range(NCORES)]
    sv_tgt = [hrows[owner == c] for c in range(NCORES)]
    SV = _build_dir_structure(sv_src, sv_tgt, cfg.RT_H // P, cfg)
    se_src = [hrows[owner == c].astype(np.int16) for c in range(NCORES)]
    se_tgt = [node_idx[owner == c] % cfg.NS for c in range(NCORES)]
    SE = _build_dir_structure(se_src, se_tgt, cfg.NT_N, cfg)

    Kw = np.asarray(inputs['Kw'], np.float32)
    Vw = np.asarray(inputs['Vw'], np.float32)
    Qw = np.asarray(inputs['Qw'], np.float32)
    W1 = np.asarray(inputs['W1'], np.float32)
    b1 = np.asarray(inputs['b1'], np.float32)
    W2 = np.asarray(inputs['W2'], np.float32)
    b2 = np.asarray(inputs['b2'], np.float32)
    g0 = np.asarray(inputs['ln0_g'], np.float32)
    be0 = np.asarray(inputs['ln0_b'], np.float32)
    g1 = np.asarray(inputs['ln1_g'], np.float32)
    be1 = np.asarray(inputs['ln1_b'], np.float32)
    NB = Kw.shape[0]
    KVw = np.zeros((NB, D, 2 * D), np.float32)
    Qrep = np.zeros((NB, P, D), np.float32)
    W1p = np.zeros((NB, D, D), np.float32)
    b1p = np.zeros((NB, D), np.float32)
    for b in range(NB):
        KVw[b, :, :D] = Kw[b].transpose(1, 0, 2).reshape(D, D)
        KVw[b, :, D:] = Vw[b].transpose(1, 0, 2).reshape(D, D)
        Qrep[b] = np.tile(Qw[b].reshape(1, D), (P, 1))
        W1p[b] = g0[b][:, None] * W1[b]
        b1p[b] = b1[b] + be0[b] @ W1[b]
        assert np.allclose(g0[b], 1.0) and np.allclose(be0[b], 0.0), \
            "non-trivial ln0 affine on residual path not emitted"
        assert np.allclose(g1[b], 1.0) and np.allclose(be1[b], 0.0), \
            "non-trivial ln1 affine not emitted"
    x0 = np.asarray(inputs['x_0'], np.float32)
    wm = dict(KVw=KVw.astype(bf), Qrep=Qrep.astype(bf),
              Qflat=Qrep.astype(np.float32), W1p=W1p.astype(bf),
              b1p=b1p.astype(np.float32), W2p=W2.astype(bf),
              b2p=b2.astype(np.float32))
    in_maps = []
    for c in range(NCORES):
        xs = np.zeros((P, cfg.NSP), bf)
        xs[:, :cfg.NS] = x0[c * cfg.NS:(c + 1) * cfg.NS].T.astype(bf)
        m = dict(xT0=xs, gidxV=SV['gidx'][c], MwV=SV['mw'][c],
                 gidxE=SE['gidx'][c], MwE=SE['mw'][c])
        m.update(wm)
        in_maps.append(m)
    return SV, SE, in_maps


def build_graph(cfg, SV, SE, n_layers=2):
    from contextlib import ExitStack
    import concourse.bacc as bacc
    import concourse.tile as tile
    from concourse import mybir
    from concourse.masks import make_identity
    from concourse import library_config

    dt = mybir.dt
    Alu = mybir.AluOpType
    Act = mybir.ActivationFunctionType
    NB = 2 * n_layers

    nc = bacc.Bacc("TRN2", target_bir_lowering=False, debug=False,
                   num_devices=NCORES, num_swdge_queues=4)
    ein, eout = "ExternalInput", "ExternalOutput"
    xT0_d = nc.dram_tensor("xT0", [P, cfg.NSP], dt.bfloat16, kind=ein)
    gV_d = nc.dram_tensor("gidxV", [P, SV['SPAD'] // 16], dt.int16, kind=ein)
    mV_d = nc.dram_tensor("MwV", list(SV['mw'][0].shape), dt.bfloat16, kind=ein)
    gE_d = nc.dram_tensor("gidxE", [P, SE['SPAD'] // 16], dt.int16, kind=ein)
    mE_d = nc.dram_tensor("MwE", list(SE['mw'][0].shape), dt.bfloat16, kind=ein)
    KVw_d = nc.dram_tensor("KVw", [NB, D, 2 * D], dt.bfloat16, kind=ein)
    Qrep_d = nc.dram_tensor("Qrep", [NB, P, D], dt.bfloat16, kind=ein)
    Qflat_d = nc.dram_tensor("Qflat", [NB, P, D], dt.float32, kind=ein)
    W1_d = nc.dram_tensor("W1p", [NB, D, D], dt.bfloat16, kind=ein)
    b1_d = nc.dram_tensor("b1p", [NB, D], dt.float32, kind=ein)
    W2_d = nc.dram_tensor("W2p", [NB, D, D], dt.bfloat16, kind=ein)
    b2_d = nc.dram_tensor("b2p", [NB, D], dt.float32, kind=ein)
    x0o_d = nc.dram_tensor("x0o", [cfg.NS, D], dt.float32, kind=eout)
    x1o_d = nc.dram_tensor("x1o", [cfg.HS, D], dt.float32, kind=eout)

    es = ExitStack()
    with tile.TileContext(nc) as tc, es:
        nc.gpsimd.load_library(library_config.mlp)
        dram = es.enter_context(tc.tile_pool(name="dram", bufs=1, space="DRAM"))
        utable_n = dram.tile([cfg.NSP, 256], dt.bfloat16)
        ushard_h = dram.tile([cfg.HSP, 256], dt.bfloat16)
        utable_hs = [dram.tile([cfg.RT_H, 256], dt.bfloat16,
                              addr_space="Shared", name=f"utable_h{l}")
                     for l in range(n_layers)]
        partial = dram.tile([cfg.RT_H, 132], dt.bfloat16)
        rsout = dram.tile([cfg.HSP, 132], dt.bfloat16)

        const = es.enter_context(tc.tile_pool(name="const", bufs=1))
        ident = const.tile([P, P], dt.bfloat16)
        make_identity(nc, ident[:])
        KVw_s = const.tile([P, NB, 2 * D], dt.bfloat16)
        nc.sync.dma_start(out=KVw_s[:], in_=KVw_d.ap().rearrange("b d m -> d b m"))
        Qrep_s = const.tile([P, NB, D], dt.bfloat16)
        nc.sync.dma_start(out=Qrep_s[:], in_=Qrep_d.ap().rearrange("b p m -> p b m"))
        Qflat_s = const.tile([P, NB, D], dt.float32)
        nc.sync.dma_start(out=Qflat_s[:], in_=Qflat_d.ap().rearrange("b p m -> p b m"))
        W1_s = const.tile([P, NB, D], dt.bfloat16)
        nc.sync.dma_start(out=W1_s[:], in_=W1_d.ap().rearrange("b d m -> d b m"))
        W2_s = const.tile([P, NB, D], dt.bfloat16)
        nc.sync.dma_start(out=W2_s[:], in_=W2_d.ap().rearrange("b d m -> d b m"))
        b1_s = const.tile([P, NB], dt.float32)
        nc.sync.dma_start(out=b1_s[:], in_=b1_d.ap().rearrange("b d -> d b"))
        b2_s = const.tile([P, NB], dt.float32)
        nc.sync.dma_start(out=b2_s[:], in_=b2_d.ap().rearrange("b d -> d b"))
        gV_s = const.tile([P, SV['SPAD'] // 16], dt.int16)
        nc.sync.dma_start(out=gV_s[:], in_=gV_d.ap())
        gE_s = const.tile([P, SE['SPAD'] // 16], dt.int16)
        nc.sync.dma_start(out=gE_s[:], in_=gE_d.ap())

        state = es.enter_context(tc.tile_pool(name="state", bufs=1))
        xT_n = state.tile([P, cfg.NSP], dt.bfloat16)
        xT_h = state.tile([P, cfg.HSP], dt.bfloat16)
        nc.sync.dma_start(out=xT_n[:], in_=xT0_d.ap())

        psum = es.enter_context(tc.tile_pool(name="psum", bufs=2, space="PSUM"))
        work = es.enter_context(tc.tile_pool(name="work", bufs=2))
        big = es.enter_context(tc.tile_pool(name="big", bufs=1))

        def source_side(xT, ntiles, b, table):
            for i0 in range(0, ntiles, 8):
                ng = min(8, ntiles - i0)
                ub = work.tile([P, 8, 256], dt.bfloat16, tag="ub", name="ub", bufs=3)
                nc.vector.memset(ub[:, :, D + H:], 0)
                for k in range(ng):
                    i = i0 + k
                    kvps = psum.tile([P, 2 * D], dt.float32, tag="kvps", name="kvps")
                    nc.tensor.matmul(out=kvps[:], lhsT=xT[:, i * P:(i + 1) * P],
                                     rhs=KVw_s[:, b:b + 1, :].squeeze(1),
                                     start=True, stop=True)
                    lgt = work.tile([P, D], dt.float32, tag="lgt", name="lgt", bufs=4)
                    nc.vector.tensor_tensor(out=lgt[:], in0=kvps[:, 0:D],
                                            in1=Qrep_s[:, b:b + 1, :].squeeze(1),
                                            op=Alu.mult)
                    lg4 = work.tile([P, H], dt.float32, tag="lg4", name="lg4", bufs=4)
                    nc.vector.tensor_reduce(
                        out=lg4[:], in_=lgt[:].rearrange("p (h k) -> p h k", h=H),
                        axis=mybir.AxisListType.X, op=Alu.add)
                    ex4 = work.tile([P, H], dt.float32, tag="ex4", name="ex4", bufs=4)
                    nc.scalar.activation(out=ex4[:], in_=lg4[:], func=Act.Exp)
                    nc.vector.tensor_tensor(
                        out=ub[:, k, 0:D].rearrange("p (h k) -> p h k", h=H),
                        in0=kvps[:, D:2 * D].rearrange("p (h k) -> p h k", h=H),
                        in1=ex4[:].unsqueeze(2).to_broadcast([P, H, KD]),
                        op=Alu.mult)
                    nc.scalar.copy(out=ub[:, k, D:D + H], in_=ex4[:])
                nc.sync.dma_start(
                    out=table[i0 * P:(i0 + ng) * P, :].rearrange(
                        "(t p) d -> p t d", p=P),
                    in_=ub[:, 0:ng, :])

        import itertools
        _gq = itertools.count()

        def gather_reduce(table, gidx_s, mw_d, S, consume):
            T = S['T']
            tb_of = S['tb_of_tile']
            mpsums = {}
            for g in range((T + cfg.CH - 1) // cfg.CH):
                t0 = g * cfg.CH
                tch = min(cfg.CH, T - t0)
                nsl = tch * P
                gbuf = work.tile([P, cfg.CH, 256], dt.bfloat16, tag="gbuf",
                                 name="gbuf", bufs=4)
                nc.gpsimd.dma_gather(
                    gbuf[:, 0:tch, :], table[:, :],
                    gidx_s[:, t0 * 8:t0 * 8 + nsl // 16], nsl, nsl, 256,
                    queue_num=next(_gq) % 4)
                mbuf = work.tile([P, cfg.CH, P], dt.bfloat16, tag="mbuf",
                                 name="mbuf", bufs=4)
                nc.sync.dma_start(
                    out=mbuf[:, 0:tch, :],
                    in_=mw_d.ap()[g:g + 1, :, 0:tch, :].squeeze(0))
                for tl in range(tch):
                    t = t0 + tl
                    tb = int(tb_of[t])
                    first = (t == 0) or (tb_of[t - 1] != tb)
                    last = (t == T - 1) or (tb_of[t + 1] != tb)
                    if first:
                        mpsums[tb] = psum.tile([P, 132], dt.float32, tag="mpsum", name="mpsum")
                    nc.tensor.matmul(out=mpsums[tb][:],
                                     lhsT=mbuf[:, tl:tl + 1, :].squeeze(1),
                                     rhs=gbuf[:, tl:tl + 1, 0:132].squeeze(1),
                                     start=first, stop=last)
                    if last:
                        consume(tb, mpsums.pop(tb))

        def flush_rows(out_dram, buf, f0, n, rows_real):
            """DMA buf[:,0:n,:] (tile-major) to out_dram rows starting f0*128,
            clipping to rows_real."""
            r0 = f0 * P
            nfull = min(n, max(0, (rows_real - r0) // P))
            if nfull > 0:
                nc.sync.dma_start(
                    out=out_dram[r0:r0 + nfull * P, :].rearrange(
                        "(t p) d -> p t d", p=P),
                    in_=buf[:, 0:nfull, :])
            rem_r = r0 + nfull * P
            rem = rows_real - rem_r
            if 0 < rem < P and nfull < n:
                nc.sync.dma_start(out=out_dram[rem_r:rem_r + rem, :],
                                  in_=buf[0:rem, nfull:nfull + 1, :].squeeze(1))

        def target_side(agg_sb, Tt, b, xT_dest, out_dram, rows_real,
                        tile_off=0):
            for G0 in range(0, Tt, cfg.TG):
                TGn = min(cfg.TG, Tt - G0)
                NCOL = TGn * D
                asl = agg_sb[:, G0:G0 + TGn, :]
                dc = work.tile([P, cfg.TG, H], dt.float32, tag="dc", name="dc")
                nc.vector.tensor_scalar(out=dc[:, 0:TGn, :],
                                        in0=asl[:, :, D:D + H],
                                        scalar1=1e-20, scalar2=None, op0=Alu.max)
                rd = work.tile([P, cfg.TG, H], dt.float32, tag="rd", name="rd")
                nc.vector.reciprocal(out=rd[:, 0:TGn, :], in_=dc[:, 0:TGn, :])
                xq = big.tile([P, cfg.TG, D], dt.float32, tag="xq", name="xq")
                nc.vector.tensor_tensor(
                    out=xq[:, 0:TGn, :].rearrange("p t (h k) -> p t h k", h=H),
                    in0=asl[:, :, 0:D].rearrange("p t (h k) -> p t h k", h=H),
                    in1=rd[:, 0:TGn, :].unsqueeze(3).to_broadcast([P, TGn, H, KD]),
                    op=Alu.mult)
                l0s = work.tile([P, cfg.TG], dt.float32, tag="l0s", name="l0s")
                for t in range(TGn):
                    nc.vector.scalar_tensor_tensor(
                        out=xq[:, t, :], in0=xq[:, t, :], scalar=1.0,
                        in1=Qflat_s[:, b, :], op0=Alu.mult, op1=Alu.add,
                        accum_out=l0s[:, t:t + 1])

                def ln_stats(src_tiles, ssum, tag, dve_sq=False):
                    # per-tile square with accum_out gives row sum-of-squares
                    s2 = work.tile([P, cfg.TG], dt.float32, tag=tag + "2")
                    for t in range(TGn):
                        sqscr = work.tile([P, D], dt.float32, tag="sqscr",
                                          name="sqscr")
                        if dve_sq:
                            st = src_tiles(t)
                            nc.vector.scalar_tensor_tensor(
                                out=sqscr[:], in0=st, scalar=1.0, in1=st,
                                op0=Alu.mult, op1=Alu.mult,
                                accum_out=s2[:, t:t + 1])
                        else:
                            nc.scalar.activation(out=sqscr[:], in_=src_tiles(t),
                                                 func=Act.Square,
                                                 accum_out=s2[:, t:t + 1])
                    mu = work.tile([P, cfg.TG], dt.float32, tag=tag + "m")
                    nc.vector.tensor_scalar(out=mu[:, 0:TGn], in0=ssum[:, 0:TGn],
                                            scalar1=1.0 / D, scalar2=None,
                                            op0=Alu.mult)
                    var = work.tile([P, cfg.TG], dt.float32, tag=tag + "v")
                    nc.vector.tensor_scalar(out=var[:, 0:TGn], in0=s2[:, 0:TGn],
                                            scalar1=1.0 / D, scalar2=None,
                                            op0=Alu.mult)
                    mu2 = work.tile([P, cfg.TG], dt.float32, tag=tag + "q")
                    nc.vector.tensor_tensor(out=mu2[:, 0:TGn], in0=mu[:, 0:TGn],
                                            in1=mu[:, 0:TGn], op=Alu.mult)
                    nc.vector.tensor_tensor(out=var[:, 0:TGn], in0=var[:, 0:TGn],
                                            in1=mu2[:, 0:TGn], op=Alu.subtract)
                    sd = work.tile([P, cfg.TG], dt.float32, tag=tag + "d")
                    nc.vector.tensor_scalar(out=sd[:, 0:TGn], in0=var[:, 0:TGn],
                                            scalar1=LN_EPS, scalar2=None,
                                            op0=Alu.add)
                    nc.scalar.sqrt(out=sd[:, 0:TGn], in_=sd[:, 0:TGn])
                    rsq = work.tile([P, cfg.TG], dt.float32, tag=tag + "r")
                    nc.vector.reciprocal(out=rsq[:, 0:TGn], in_=sd[:, 0:TGn])
                    nm = work.tile([P, cfg.TG], dt.float32, tag=tag + "n")
                    nc.vector.tensor_tensor(out=nm[:, 0:TGn], in0=mu[:, 0:TGn],
                                            in1=rsq[:, 0:TGn], op=Alu.mult)
                    nc.vector.tensor_scalar(out=nm[:, 0:TGn], in0=nm[:, 0:TGn],
                                            scalar1=-1.0, scalar2=None,
                                            op0=Alu.mult)
                    return rsq, nm

                rsq0, nm0 = ln_stats(lambda t: xq[:, t, :], l0s, "l0", dve_sq=True)
                xh = big.tile([P, cfg.TG, D], dt.bfloat16, tag="xh", name="xh")
                for t in range(TGn):
                    nc.vector.scalar_tensor_tensor(
                        out=xh[:, t, :], in0=xq[:, t, :],
                        scalar=rsq0[:, t:t + 1],
                        in1=nm0[:, t:t + 1].to_broadcast([P, D]),
                        op0=Alu.mult, op1=Alu.add)
                xhT = big.tile([P, cfg.TG, D], dt.bfloat16, tag="xhT", name="xhT")
                for t in range(TGn):
                    tps = psum.tile([P, P], dt.bfloat16, tag="tps", name="tps")
                    nc.tensor.transpose(out=tps[:],
                                        in_=xh[:, t:t + 1, :].squeeze(1),
                                        identity=ident[:])
                    nc.scalar.copy(out=xhT[:, t, :], in_=tps[:])
                h1T = big.tile([P, cfg.TG, D], dt.bfloat16, tag="h1T", name="h1T")
                xhT_f = xhT[:].rearrange("p t d -> p (t d)")
                h1T_f = h1T[:].rearrange("p t d -> p (t d)")
                for c0 in range(0, NCOL, 512):
                    cw = min(512, NCOL - c0)
                    mlp1 = psum.tile([P, 512], dt.float32, tag="mlpps", name="mlpps")
                    nc.tensor.matmul(out=mlp1[:, 0:cw],
                                     lhsT=W1_s[:, b:b + 1, :].squeeze(1),
                                     rhs=xhT_f[:, c0:c0 + cw],
                                     start=True, stop=True)
                    nc.scalar.activation(out=h1T_f[:, c0:c0 + cw],
                                         in_=mlp1[:, 0:cw], func=Act.Relu,
                                         bias=b1_s[:, b:b + 1], scale=1.0)
                h2T = big.tile([P, cfg.TG, D], dt.bfloat16, tag="h2T", name="h2T")
                h2T_f = h2T[:].rearrange("p t d -> p (t d)")
                for c0 in range(0, NCOL, 512):
                    cw = min(512, NCOL - c0)
                    mlp2 = psum.tile([P, 512], dt.float32, tag="mlpps", name="mlpps")
                    nc.tensor.matmul(out=mlp2[:, 0:cw],
                                     lhsT=W2_s[:, b:b + 1, :].squeeze(1),
                                     rhs=h1T_f[:, c0:c0 + cw],
                                     start=True, stop=True)
                    nc.scalar.activation(out=h2T_f[:, c0:c0 + cw],
                                         in_=mlp2[:, 0:cw], func=Act.Identity,
                                         bias=b2_s[:, b:b + 1], scale=1.0)
                y = big.tile([P, cfg.TG, D], dt.float32, tag="xq", name="xq")
                l1s = work.tile([P, cfg.TG], dt.float32, tag="l1s", name="l1s")
                for t in range(TGn):
                    tps2 = psum.tile([P, P], dt.bfloat16, tag="tps", name="tps")
                    nc.tensor.transpose(out=tps2[:],
                                        in_=h2T[:, t:t + 1, :].squeeze(1),
                                        identity=ident[:])
                    nc.vector.scalar_tensor_tensor(
                        out=y[:, t, :], in0=xh[:, t:t + 1, :].squeeze(1),
                        scalar=1.0, in1=tps2[:], op0=Alu.mult, op1=Alu.add,
                        accum_out=l1s[:, t:t + 1])
                rsq1, nm1 = ln_stats(lambda t: y[:, t, :], l1s, "l1")
                xob = None
                for t in range(TGn):
                    gt = tile_off + G0 + t
                    if t % 8 == 0:
                        if xob is not None:
                            flush_rows(out_dram, xob, tile_off + G0 + t - 8, 8,
                                       rows_real) \
                                if out_dram is not None else None
                        xob = work.tile([P, 8, D], dt.float32, tag="xob", name="xob")
                    nc.scalar.activation(out=xob[:, t % 8, :], in_=y[:, t:t + 1, :].squeeze(1),
                                         func=Act.Relu, scale=rsq1[:, t:t + 1],
                                         bias=nm1[:, t:t + 1])
                    xnb = work.tile([P, D], dt.bfloat16, tag="xnb", name="xnb")
                    nc.vector.tensor_copy(xnb[:], xob[:, t % 8, :])
                    tps3 = psum.tile([P, P], dt.bfloat16, tag="tps", name="tps")
                    nc.tensor.transpose(out=tps3[:], in_=xnb[:], identity=ident[:])
                    nc.vector.tensor_copy(xT_dest[:, gt * P:(gt + 1) * P], tps3[:])
                if out_dram is not None and xob is not None:
                    nlast = TGn - (TGn - 1) // 8 * 8
                    flush_rows(out_dram, xob, tile_off + G0 + TGn - nlast, nlast,
                               rows_real)

        rg = [list(range(NCORES))]
        for layer in range(n_layers):
            bv, bev = 2 * layer, 2 * layer + 1
            last = layer == n_layers - 1
            # ---- v2e ----
            source_side(xT_n, cfg.NT_N, bv, utable_n)
            pstate = {}

            def v2e_consume(tb, mp, pstate=pstate):
                if tb % 8 == 0:
                    pstate['pb'] = work.tile([P, 8, 132], dt.bfloat16, tag="pb", name="pb")
                nc.scalar.activation(out=pstate['pb'][:, tb % 8, :], in_=mp[:],
                                     func=Act.Copy)
                ntb = cfg.RT_H // P
                if tb % 8 == 7 or tb == ntb - 1:
                    n = tb % 8 + 1
                    nc.sync.dma_start(
                        out=partial[(tb - n + 1) * P:(tb + 1) * P, :].rearrange(
                            "(t p) d -> p t d", p=P),
                        in_=pstate['pb'][:, 0:n, :])
            gather_reduce(utable_n, gV_s, mV_d, SV, v2e_consume)
            nc.gpsimd.collective_compute(
                "ReduceScatter", Alu.add, replica_groups=rg,
                ins=[partial.opt()], outs=[rsout.opt()])
            agg_h = big.tile([P, cfg.NT_H, 132], dt.bfloat16, tag="aggh", name="aggh")
            nc.sync.dma_start(out=agg_h[:],
                              in_=rsout[:, :].rearrange("(t p) d -> p t d", p=P))
            target_side(agg_h, cfg.NT_H, bv, xT_h,
                        x1o_d.ap() if last else None, cfg.HS)
            # ---- e2v ----
            source_side(xT_h, cfg.NT_H, bev, ushard_h)
            utable_h = utable_hs[layer]
            nc.gpsimd.collective_compute(
                "AllGather", Alu.bypass, replica_groups=rg,
                ins=[ushard_h.opt()], outs=[utable_h.opt()])
            ngr = (cfg.NT_N + cfg.TG - 1) // cfg.TG
            agg_n_g = [big.tile([P, min(cfg.TG, cfg.NT_N - gi * cfg.TG), 132],
                               dt.bfloat16, tag=f"aggn{gi}", name=f"aggn{gi}")
                       for gi in range(ngr)]
            e2v_out = x0o_d.ap() if last else None

            def e2v_consume(tb, mp, agg_n_g=agg_n_g, bev=bev, e2v_out=e2v_out):
                gi = tb // cfg.TG
                nc.scalar.activation(out=agg_n_g[gi][:, tb % cfg.TG, :],
                                     in_=mp[:], func=Act.Copy)
                if tb == min((gi + 1) * cfg.TG, cfg.NT_N) - 1:
                    target_side(agg_n_g[gi], agg_n_g[gi].shape[1], bev, xT_n,
                                e2v_out, cfg.NS, tile_off=gi * cfg.TG)
            gather_reduce(utable_h, gE_s, mE_d, SE, e2v_consume)

    nc.compile()
    return nc


def build(inputs, cfg=None):
    cfg = cfg or Cfg(100000, 30000, 400000)
    SV, SE, in_maps = preprocess(inputs, cfg)
    nc = build_graph(cfg, SV, SE)
    return nc, in_maps


def execute(nc, in_maps, trace=False):
    from concourse.bass_utils import run_bass_kernel_spmd
    res = run_bass_kernel_spmd(nc, in_maps, core_ids=list(range(NCORES)),
                               trace=trace)
    x0 = np.concatenate([np.asarray(res.results[c]['x0o']) for c in range(NCORES)], 0)
    x1 = np.concatenate([np.asarray(res.results[c]['x1o']) for c in range(NCORES)], 0)
    return (x0.astype(np.float32), x1.astype(np.float32)), res


def run(inputs, cfg=None, trace=False):
    nc, in_maps = build(inputs, cfg)
    return execute(nc, in_maps, trace=trace)


def kernel(**inputs):
    (x0, x1), _ = run(inputs)
    return (x0, x1)


# revision 22
# speedup vs baseline: 1.2264x; 1.0966x over previous
"""AllSetTransformer hypergraph network on 8 TRN2 NeuronCores.

Sharding: nodes 12500/core, hyperedges 3750/core (padded-hedge global rows
8x3840=30720, which fits int16 for dma_gather).
  v2e blocks: per-core local U=[ex*Vx|ex] table (own node shard), chunked
    dma_gather with local int16 indices, one-hot M' matmuls reduce pin rows
    into PSUM partials over ALL hyperedge rows, bf16 ReduceScatter combines.
  e2v blocks: U table for hedge shard -> AllGather (full table), dma_gather
    with global padded-hedge idx, M' matmuls into local node-shard targets.
Target side: denom clamp+divide, +Q, LN0 (affine folded into W1/b1), MLP in
transposed layout (PE transposes), residual, LN1, relu.
Softmax max-subtraction skipped (shift-invariant, logits are O(1)).
"""
import sys
if '/opt/trn_rl_repo' not in sys.path:
    sys.path.insert(0, '/opt/trn_rl_repo')
import numpy as np
import ml_dtypes

bf = ml_dtypes.bfloat16

D, H, KD = 128, 4, 32
NCORES = 8
LN_EPS = 1e-5
P = 128


def _pad128(n):
    return ((n + 127) // 128) * 128


class Cfg:
    def __init__(self, NN, NH, E, chunk_tiles=8):
        self.NN, self.NH, self.E = NN, NH, E
        assert NN % NCORES == 0 and NH % NCORES == 0
        self.NS, self.HS = NN // NCORES, NH // NCORES
        self.NSP, self.HSP = _pad128(self.NS), _pad128(self.HS)
        self.NT_N, self.NT_H = self.NSP // P, self.HSP // P
        self.RT_H = NCORES * self.HSP      # padded global hedge rows
        assert self.RT_H <= 32767
        self.CH = chunk_tiles              # gather chunk size in 128-slot tiles
        self.TG = 16                       # target-side group in 128-row tiles


def _build_dir_structure(src_local, tgt_row, n_tb, cfg):
    """SPMD-shared tile structure + per-core gather idx / M' arrays."""
    per_core = []
    counts = np.zeros((NCORES, n_tb), np.int64)
    for c in range(NCORES):
        tb = tgt_row[c] // P
        order = np.argsort(tb, kind='stable')
        sl, tr = src_local[c][order], tgt_row[c][order]
        counts[c] = np.bincount(tb[order], minlength=n_tb)
        per_core.append((sl, tr))
    ntiles = np.maximum(1, (counts.max(0) + P - 1) // P).astype(np.int64)
    T = int(ntiles.sum())
    tb_of_tile = np.repeat(np.arange(n_tb), ntiles)
    tile_base = np.concatenate([[0], np.cumsum(ntiles)])
    S = T * P
    SPAD = ((S + cfg.CH * P - 1) // (cfg.CH * P)) * (cfg.CH * P)
    gidxs, mws = [], []
    for c in range(NCORES):
        sl, tr = per_core[c]
        gidx = np.zeros(SPAD, np.int16)
        mw = np.zeros((T, P, P), np.float32)
        starts = np.concatenate([[0], np.cumsum(counts[c])])
        for j in range(n_tb):
            cnt = int(counts[c][j])
            if cnt == 0:
                continue
            s0 = int(tile_base[j]) * P
            a = int(starts[j])
            slots = np.arange(s0, s0 + cnt)
            gidx[slots] = sl[a:a + cnt]
            tl = (tr[a:a + cnt] - j * P).astype(np.int64)
            np.add.at(mw, (slots // P, slots % P, tl), 1.0)
        packed = np.ascontiguousarray(gidx.reshape(-1, 16).T)   # [16, SPAD/16]
        gidxs.append(np.tile(packed, (8, 1)))                   # [128, SPAD/16]
        # pack M' chunk-contiguous: [nch, P, CH, P] so each chunk DMA is linear
        nch = (T + cfg.CH - 1) // cfg.CH
        mwp = np.zeros((nch, P, cfg.CH, P), np.float32)
        for g in range(nch):
            blk = mw[g * cfg.CH:(g + 1) * cfg.CH]
            mwp[g, :, :blk.shape[0], :] = blk.transpose(1, 0, 2)
        mws.append(mwp.astype(bf))
    return dict(ntiles=ntiles, tb_of_tile=tb_of_tile, T=T, S=S, SPAD=SPAD,
                gidx=gidxs, mw=mws)


def preprocess(inputs, cfg):
    node_idx = np.asarray(inputs['node_idx']).astype(np.int64)
    hedge_idx = np.asarray(inputs['hedge_idx']).astype(np.int64)
    owner = node_idx // cfg.NS
    hrows = (hedge_idx // cfg.HS) * cfg.HSP + (hedge_idx % cfg.HS)
    sv_src = [(node_idx[owner == c] % cfg.NS).astype(np.int16) for c in range(NCORES)]
    sv_tgt = [hrows[owner == c] for c in range(NCORES)]
    SV = _build_dir_structure(sv_src, sv_tgt, cfg.RT_H // P, cfg)
    se_src = [hrows[owner == c].astype(np.int16) for c in range(NCORES)]
    se_tgt = [node_idx[owner == c] % cfg.NS for c in range(NCORES)]
    SE = _build_dir_structure(se_src, se_tgt, cfg.NT_N, cfg)

    Kw = np.asarray(inputs['Kw'], np.float32)
    Vw = np.asarray(inputs['Vw'], np.float32)
    Qw = np.asarray(inputs['Qw'], np.float32)
    W1 = np.asarray(inputs['W1'], np.float32)
    b1 = np.asarray(inputs['b1'], np.float32)
    W2 = np.asarray(inputs['W2'], np.float32)
    b2 = np.asarray(inputs['b2'], np.float32)
    g0 = np.asarray(inputs['ln0_g'], np.float32)
    be0 = np.asarray(inputs['ln0_b'], np.float32)
    g1 = np.asarray(inputs['ln1_g'], np.float32)
    be1 = np.asarray(inputs['ln1_b'], np.float32)
    NB = Kw.shape[0]
    KVw = np.zeros((NB, D, 2 * D), np.float32)
    Qrep = np.zeros((NB, P, D), np.float32)
    W1p = np.zeros((NB, D, D), np.float32)
    b1p = np.zeros((NB, D), np.float32)
    for b in range(NB):
        KVw[b, :, :D] = Kw[b].transpose(1, 0, 2).reshape(D, D)
        KVw[b, :, D:] = Vw[b].transpose(1, 0, 2).reshape(D, D)
        Qrep[b] = np.tile(Qw[b].reshape(1, D), (P, 1))
        W1p[b] = g0[b][:, None] * W1[b]
        b1p[b] = b1[b] + be0[b] @ W1[b]
        assert np.allclose(g0[b], 1.0) and np.allclose(be0[b], 0.0), \
            "non-trivial ln0 affine on residual path not emitted"
        assert np.allclose(g1[b], 1.0) and np.allclose(be1[b], 0.0), \
            "non-trivial ln1 affine not emitted"
    x0 = np.asarray(inputs['x_0'], np.float32)
    wm = dict(KVw=KVw.astype(bf), Qrep=Qrep.astype(bf),
              Qflat=Qrep.astype(np.float32), W1p=W1p.astype(bf),
              b1p=b1p.astype(np.float32), W2p=W2.astype(bf),
              b2p=b2.astype(np.float32))
    in_maps = []
    for c in range(NCORES):
        xs = np.zeros((P, cfg.NSP), bf)
        xs[:, :cfg.NS] = x0[c * cfg.NS:(c + 1) * cfg.NS].T.astype(bf)
        m = dict(xT0=xs, gidxV=SV['gidx'][c], MwV=SV['mw'][c],
                 gidxE=SE['gidx'][c], MwE=SE['mw'][c])
        m.update(wm)
        in_maps.append(m)
    return SV, SE, in_maps


def build_graph(cfg, SV, SE, n_layers=2):
    from contextlib import ExitStack
    import concourse.bacc as bacc
    import concourse.tile as tile
    from concourse import mybir
    from concourse.masks import make_identity
    from concourse import library_config

    dt = mybir.dt
    Alu = mybir.AluOpType
    Act = mybir.ActivationFunctionType
    NB = 2 * n_layers

    nc = bacc.Bacc("TRN2", target_bir_lowering=False, debug=False,
                   num_devices=NCORES, num_swdge_queues=4)
    ein, eout = "ExternalInput", "ExternalOutput"
    xT0_d = nc.dram_tensor("xT0", [P, cfg.NSP], dt.bfloat16, kind=ein)
    gV_d = nc.dram_tensor("gidxV", [P, SV['SPAD'] // 16], dt.int16, kind=ein)
    mV_d = nc.dram_tensor("MwV", list(SV['mw'][0].shape), dt.bfloat16, kind=ein)
    gE_d = nc.dram_tensor("gidxE", [P, SE['SPAD'] // 16], dt.int16, kind=ein)
    mE_d = nc.dram_tensor("MwE", list(SE['mw'][0].shape), dt.bfloat16, kind=ein)
    KVw_d = nc.dram_tensor("KVw", [NB, D, 2 * D], dt.bfloat16, kind=ein)
    Qrep_d = nc.dram_tensor("Qrep", [NB, P, D], dt.bfloat16, kind=ein)
    Qflat_d = nc.dram_tensor("Qflat", [NB, P, D], dt.float32, kind=ein)
    W1_d = nc.dram_tensor("W1p", [NB, D, D], dt.bfloat16, kind=ein)
    b1_d = nc.dram_tensor("b1p", [NB, D], dt.float32, kind=ein)
    W2_d = nc.dram_tensor("W2p", [NB, D, D], dt.bfloat16, kind=ein)
    b2_d = nc.dram_tensor("b2p", [NB, D], dt.float32, kind=ein)
    x0o_d = nc.dram_tensor("x0o", [cfg.NS, D], dt.float32, kind=eout)
    x1o_d = nc.dram_tensor("x1o", [cfg.HS, D], dt.float32, kind=eout)

    es = ExitStack()
    with tile.TileContext(nc) as tc, es:
        nc.gpsimd.load_library(library_config.mlp)
        dram = es.enter_context(tc.tile_pool(name="dram", bufs=1, space="DRAM"))
        utable_n = dram.tile([cfg.NSP, 256], dt.bfloat16)
        ushard_h = dram.tile([cfg.HSP, 256], dt.bfloat16)
        utable_hs = [dram.tile([cfg.RT_H, 256], dt.bfloat16,
                              addr_space="Shared", name=f"utable_h{l}")
                     for l in range(n_layers)]
        partial = dram.tile([cfg.RT_H, 132], dt.bfloat16)
        rsout = dram.tile([cfg.HSP, 132], dt.bfloat16)

        const = es.enter_context(tc.tile_pool(name="const", bufs=1))
        ident = const.tile([P, P], dt.bfloat16)
        make_identity(nc, ident[:])
        KVw_s = const.tile([P, NB, 2 * D], dt.bfloat16)
        nc.sync.dma_start(out=KVw_s[:], in_=KVw_d.ap().rearrange("b d m -> d b m"))
        Qrep_s = const.tile([P, NB, D], dt.bfloat16)
        nc.sync.dma_start(out=Qrep_s[:], in_=Qrep_d.ap().rearrange("b p m -> p b m"))
        Qflat_s = const.tile([P, NB, D], dt.float32)
        nc.sync.dma_start(out=Qflat_s[:], in_=Qflat_d.ap().rearrange("b p m -> p b m"))
        W1_s = const.tile([P, NB, D], dt.bfloat16)
        nc.sync.dma_start(out=W1_s[:], in_=W1_d.ap().rearrange("b d m -> d b m"))
        W2_s = const.tile([P, NB, D], dt.bfloat16)
        nc.sync.dma_start(out=W2_s[:], in_=W2_d.ap().rearrange("b d m -> d b m"))
        b1_s = const.tile([P, NB], dt.float32)
        nc.sync.dma_start(out=b1_s[:], in_=b1_d.ap().rearrange("b d -> d b"))
        b2_s = const.tile([P, NB], dt.float32)
        nc.sync.dma_start(out=b2_s[:], in_=b2_d.ap().rearrange("b d -> d b"))
        gV_s = const.tile([P, SV['SPAD'] // 16], dt.int16)
        nc.sync.dma_start(out=gV_s[:], in_=gV_d.ap())
        gE_s = const.tile([P, SE['SPAD'] // 16], dt.int16)
        nc.sync.dma_start(out=gE_s[:], in_=gE_d.ap())

        state = es.enter_context(tc.tile_pool(name="state", bufs=1))
        xT_n = state.tile([P, cfg.NSP], dt.bfloat16)
        xT_h = state.tile([P, cfg.HSP], dt.bfloat16)
        nc.sync.dma_start(out=xT_n[:], in_=xT0_d.ap())

        psum = es.enter_context(tc.tile_pool(name="psum", bufs=2, space="PSUM"))
        work = es.enter_context(tc.tile_pool(name="work", bufs=2))
        big = es.enter_context(tc.tile_pool(name="big", bufs=1))

        def source_side(xT, ntiles, b, table):
            for i0 in range(0, ntiles, 8):
                ng = min(8, ntiles - i0)
                ub = work.tile([P, 8, 256], dt.bfloat16, tag="ub", name="ub", bufs=3)
                nc.vector.memset(ub[:, :, D + H:], 0)
                for k in range(ng):
                    i = i0 + k
                    kvps = psum.tile([P, 2 * D], dt.float32, tag="kvps", name="kvps")
                    nc.tensor.matmul(out=kvps[:], lhsT=xT[:, i * P:(i + 1) * P],
                                     rhs=KVw_s[:, b:b + 1, :].squeeze(1),
                                     start=True, stop=True)
                    lgt = work.tile([P, D], dt.float32, tag="lgt", name="lgt", bufs=4)
                    nc.vector.tensor_tensor(out=lgt[:], in0=kvps[:, 0:D],
                                            in1=Qrep_s[:, b:b + 1, :].squeeze(1),
                                            op=Alu.mult)
                    lg4 = work.tile([P, H], dt.float32, tag="lg4", name="lg4", bufs=4)
                    nc.vector.tensor_reduce(
                        out=lg4[:], in_=lgt[:].rearrange("p (h k) -> p h k", h=H),
                        axis=mybir.AxisListType.X, op=Alu.add)
                    ex4 = work.tile([P, H], dt.float32, tag="ex4", name="ex4", bufs=4)
                    nc.scalar.activation(out=ex4[:], in_=lg4[:], func=Act.Exp)
                    nc.vector.tensor_tensor(
                        out=ub[:, k, 0:D].rearrange("p (h k) -> p h k", h=H),
                        in0=kvps[:, D:2 * D].rearrange("p (h k) -> p h k", h=H),
                        in1=ex4[:].unsqueeze(2).to_broadcast([P, H, KD]),
                        op=Alu.mult)
                    nc.scalar.copy(out=ub[:, k, D:D + H], in_=ex4[:])
                nc.sync.dma_start(
                    out=table[i0 * P:(i0 + ng) * P, :].rearrange(
                        "(t p) d -> p t d", p=P),
                    in_=ub[:, 0:ng, :])

        import itertools
        _gq = itertools.count()

        def gather_reduce(table, gidx_s, mw_d, S, consume):
            T = S['T']
            tb_of = S['tb_of_tile']
            mpsums = {}
            for g in range((T + cfg.CH - 1) // cfg.CH):
                t0 = g * cfg.CH
                tch = min(cfg.CH, T - t0)
                nsl = tch * P
                gbuf = work.tile([P, cfg.CH, 256], dt.bfloat16, tag="gbuf",
                                 name="gbuf", bufs=4)
                nc.gpsimd.dma_gather(
                    gbuf[:, 0:tch, :], table[:, :],
                    gidx_s[:, t0 * 8:t0 * 8 + nsl // 16], nsl, nsl, 256,
                    queue_num=next(_gq) % 4)
                mbuf = work.tile([P, cfg.CH, P], dt.bfloat16, tag="mbuf",
                                 name="mbuf", bufs=4)
                nc.sync.dma_start(
                    out=mbuf[:, 0:tch, :],
                    in_=mw_d.ap()[g:g + 1, :, 0:tch, :].squeeze(0))
                for tl in range(tch):
                    t = t0 + tl
                    tb = int(tb_of[t])
                    first = (t == 0) or (tb_of[t - 1] != tb)
                    last = (t == T - 1) or (tb_of[t + 1] != tb)
                    if first:
                        mpsums[tb] = psum.tile([P, 132], dt.float32, tag="mpsum", name="mpsum")
                    nc.tensor.matmul(out=mpsums[tb][:],
                                     lhsT=mbuf[:, tl:tl + 1, :].squeeze(1),
                                     rhs=gbuf[:, tl:tl + 1, 0:132].squeeze(1),
                                     start=first, stop=last)
                    if last:
                        consume(tb, mpsums.pop(tb))

        def flush_rows(out_dram, buf, f0, n, rows_real):
            """DMA buf[:,0:n,:] (tile-major) to out_dram rows starting f0*128,
            clipping to rows_real."""
            r0 = f0 * P
            nfull = min(n, max(0, (rows_real - r0) // P))
            if nfull > 0:
                nc.sync.dma_start(
                    out=out_dram[r0:r0 + nfull * P, :].rearrange(
                        "(t p) d -> p t d", p=P),
                    in_=buf[:, 0:nfull, :])
            rem_r = r0 + nfull * P
            rem = rows_real - rem_r
            if 0 < rem < P and nfull < n:
                nc.sync.dma_start(out=out_dram[rem_r:rem_r + rem, :],
                                  in_=buf[0:rem, nfull:nfull + 1, :].squeeze(1))

        def target_side(agg_sb, Tt, b, xT_dest, out_dram, rows_real,
                        tile_off=0):
            for G0 in range(0, Tt, cfg.TG):
                TGn = min(cfg.TG, Tt - G0)
                NCOL = TGn * D
                asl = agg_sb[:, G0:G0 + TGn, :]
                dc = work.tile([P, cfg.TG, H], dt.float32, tag="dc", name="dc")
                nc.vector.tensor_scalar(out=dc[:, 0:TGn, :],
                                        in0=asl[:, :, D:D + H],
                                        scalar1=1e-20, scalar2=None, op0=Alu.max)
                rd = work.tile([P, cfg.TG, H], dt.float32, tag="rd", name="rd")
                nc.vector.reciprocal(out=rd[:, 0:TGn, :], in_=dc[:, 0:TGn, :])
                xq = big.tile([P, cfg.TG, D], dt.float32, tag="xq", name="xq")
                nc.vector.tensor_tensor(
                    out=xq[:, 0:TGn, :].rearrange("p t (h k) -> p t h k", h=H),
                    in0=asl[:, :, 0:D].rearrange("p t (h k) -> p t h k", h=H),
                    in1=rd[:, 0:TGn, :].unsqueeze(3).to_broadcast([P, TGn, H, KD]),
                    op=Alu.mult)
                l0s = work.tile([P, cfg.TG], dt.float32, tag="l0s", name="l0s")
                for t in range(TGn):
                    nc.vector.scalar_tensor_tensor(
                        out=xq[:, t, :], in0=xq[:, t, :], scalar=1.0,
                        in1=Qflat_s[:, b, :], op0=Alu.mult, op1=Alu.add,
                        accum_out=l0s[:, t:t + 1])

                def ln_stats(src_tiles, ssum, tag):
                    # per-tile Square with accum_out gives row sum-of-squares
                    s2 = work.tile([P, cfg.TG], dt.float32, tag=tag + "2")
                    for t in range(TGn):
                        sqscr = work.tile([P, D], dt.float32, tag="sqscr",
                                          name="sqscr")
                        nc.scalar.activation(out=sqscr[:], in_=src_tiles(t),
                                             func=Act.Square,
                                             accum_out=s2[:, t:t + 1])
                    mu = work.tile([P, cfg.TG], dt.float32, tag=tag + "m")
                    nc.vector.tensor_scalar(out=mu[:, 0:TGn], in0=ssum[:, 0:TGn],
                                            scalar1=1.0 / D, scalar2=None,
                                            op0=Alu.mult)
                    var = work.tile([P, cfg.TG], dt.float32, tag=tag + "v")
                    nc.vector.tensor_scalar(out=var[:, 0:TGn], in0=s2[:, 0:TGn],
                                            scalar1=1.0 / D, scalar2=None,
                                            op0=Alu.mult)
                    mu2 = work.tile([P, cfg.TG], dt.float32, tag=tag + "q")
                    nc.vector.tensor_tensor(out=mu2[:, 0:TGn], in0=mu[:, 0:TGn],
                                            in1=mu[:, 0:TGn], op=Alu.mult)
                    nc.vector.tensor_tensor(out=var[:, 0:TGn], in0=var[:, 0:TGn],
                                            in1=mu2[:, 0:TGn], op=Alu.subtract)
                    sd = work.tile([P, cfg.TG], dt.float32, tag=tag + "d")
                    nc.vector.tensor_scalar(out=sd[:, 0:TGn], in0=var[:, 0:TGn],
                                            scalar1=LN_EPS, scalar2=None,
                                            op0=Alu.add)
                    nc.scalar.sqrt(out=sd[:, 0:TGn], in_=sd[:, 0:TGn])
                    rsq = work.tile([P, cfg.TG], dt.float32, tag=tag + "r")
                    nc.vector.reciprocal(out=rsq[:, 0:TGn], in_=sd[:, 0:TGn])
                    nm = work.tile([P, cfg.TG], dt.float32, tag=tag + "n")
                    nc.vector.tensor_tensor(out=nm[:, 0:TGn], in0=mu[:, 0:TGn],
                                            in1=rsq[:, 0:TGn], op=Alu.mult)
                    nc.vector.tensor_scalar(out=nm[:, 0:TGn], in0=nm[:, 0:TGn],
                                            scalar1=-1.0, scalar2=None,
                                            op0=Alu.mult)
                    return rsq, nm

                rsq0, nm0 = ln_stats(lambda t: xq[:, t, :], l0s, "l0")
                xh = big.tile([P, cfg.TG, D], dt.bfloat16, tag="xh", name="xh")
                for t in range(TGn):
                    nc.vector.scalar_tensor_tensor(
                        out=xh[:, t, :], in0=xq[:, t, :],
                        scalar=rsq0[:, t:t + 1],
                        in1=nm0[:, t:t + 1].to_broadcast([P, D]),
                        op0=Alu.mult, op1=Alu.add)
                xhT = big.tile([P, cfg.TG, D], dt.bfloat16, tag="xhT", name="xhT")
                for t in range(TGn):
                    tps = psum.tile([P, P], dt.bfloat16, tag="tps", name="tps")
                    nc.tensor.transpose(out=tps[:],
                                        in_=xh[:, t:t + 1, :].squeeze(1),
                                        identity=ident[:])
                    nc.scalar.copy(out=xhT[:, t, :], in_=tps[:])
                h1T = big.tile([P, cfg.TG, D], dt.bfloat16, tag="h1T", name="h1T")
                xhT_f = xhT[:].rearrange("p t d -> p (t d)")
                h1T_f = h1T[:].rearrange("p t d -> p (t d)")
                for c0 in range(0, NCOL, 512):
                    cw = min(512, NCOL - c0)
                    mlp1 = psum.tile([P, 512], dt.float32, tag="mlpps", name="mlpps")
                    nc.tensor.matmul(out=mlp1[:, 0:cw],
                                     lhsT=W1_s[:, b:b + 1, :].squeeze(1),
                                     rhs=xhT_f[:, c0:c0 + cw],
                                     start=True, stop=True)
                    nc.scalar.activation(out=h1T_f[:, c0:c0 + cw],
                                         in_=mlp1[:, 0:cw], func=Act.Relu,
                                         bias=b1_s[:, b:b + 1], scale=1.0)
                h2T = big.tile([P, cfg.TG, D], dt.bfloat16, tag="h2T", name="h2T")
                h2T_f = h2T[:].rearrange("p t d -> p (t d)")
                for c0 in range(0, NCOL, 512):
                    cw = min(512, NCOL - c0)
                    mlp2 = psum.tile([P, 512], dt.float32, tag="mlpps", name="mlpps")
                    nc.tensor.matmul(out=mlp2[:, 0:cw],
                                     lhsT=W2_s[:, b:b + 1, :].squeeze(1),
                                     rhs=h1T_f[:, c0:c0 + cw],
                                     start=True, stop=True)
                    nc.scalar.activation(out=h2T_f[:, c0:c0 + cw],
                                         in_=mlp2[:, 0:cw], func=Act.Identity,
                                         bias=b2_s[:, b:b + 1], scale=1.0)
                y = big.tile([P, cfg.TG, D], dt.float32, tag="xq", name="xq")
                l1s = work.tile([P, cfg.TG], dt.float32, tag="l1s", name="l1s")
                for t in range(TGn):
                    tps2 = psum.tile([P, P], dt.bfloat16, tag="tps", name="tps")
                    nc.tensor.transpose(out=tps2[:],
                                        in_=h2T[:, t:t + 1, :].squeeze(1),
                                        identity=ident[:])
                    nc.vector.scalar_tensor_tensor(
                        out=y[:, t, :], in0=xh[:, t:t + 1, :].squeeze(1),
                        scalar=1.0, in1=tps2[:], op0=Alu.mult, op1=Alu.add,
                        accum_out=l1s[:, t:t + 1])
                rsq1, nm1 = ln_stats(lambda t: y[:, t, :], l1s, "l1")
                xob = None
                for t in range(TGn):
                    gt = tile_off + G0 + t
                    if t % 8 == 0:
                        if xob is not None:
                            flush_rows(out_dram, xob, tile_off + G0 + t - 8, 8,
                                       rows_real) \
                                if out_dram is not None else None
                        xob = work.tile([P, 8, D], dt.float32, tag="xob", name="xob")
                    nc.scalar.activation(out=xob[:, t % 8, :], in_=y[:, t:t + 1, :].squeeze(1),
                                         func=Act.Relu, scale=rsq1[:, t:t + 1],
                                         bias=nm1[:, t:t + 1])
                    xnb = work.tile([P, D], dt.bfloat16, tag="xnb", name="xnb")
                    nc.scalar.copy(out=xnb[:], in_=xob[:, t % 8, :])
                    tps3 = psum.tile([P, P], dt.bfloat16, tag="tps", name="tps")
                    nc.tensor.transpose(out=tps3[:], in_=xnb[:], identity=ident[:])
                    nc.vector.tensor_copy(xT_dest[:, gt * P:(gt + 1) * P], tps3[:])
                if out_dram is not None and xob is not None:
                    nlast = TGn - (TGn - 1) // 8 * 8
                    flush_rows(out_dram, xob, tile_off + G0 + TGn - nlast, nlast,
                               rows_real)

        rg = [list(range(NCORES))]
        for layer in range(n_layers):
            bv, bev = 2 * layer, 2 * layer + 1
            last = layer == n_layers - 1
            # ---- v2e ----
            source_side(xT_n, cfg.NT_N, bv, utable_n)
            pstate = {}

            def v2e_consume(tb, mp, pstate=pstate):
                if tb % 8 == 0:
                    pstate['pb'] = work.tile([P, 8, 132], dt.bfloat16, tag="pb", name="pb")
                nc.scalar.activation(out=pstate['pb'][:, tb % 8, :], in_=mp[:],
                                     func=Act.Copy)
                ntb = cfg.RT_H // P
                if tb % 8 == 7 or tb == ntb - 1:
                    n = tb % 8 + 1
                    nc.sync.dma_start(
                        out=partial[(tb - n + 1) * P:(tb + 1) * P, :].rearrange(
                            "(t p) d -> p t d", p=P),
                        in_=pstate['pb'][:, 0:n, :])
            gather_reduce(utable_n, gV_s, mV_d, SV, v2e_consume)
            nc.gpsimd.collective_compute(
                "ReduceScatter", Alu.add, replica_groups=rg,
                ins=[partial.opt()], outs=[rsout.opt()])
            agg_h = big.tile([P, cfg.NT_H, 132], dt.bfloat16, tag="aggh", name="aggh")
            nc.sync.dma_start(out=agg_h[:],
                              in_=rsout[:, :].rearrange("(t p) d -> p t d", p=P))
            target_side(agg_h, cfg.NT_H, bv, xT_h,
                        x1o_d.ap() if last else None, cfg.HS)
            # ---- e2v ----
            source_side(xT_h, cfg.NT_H, bev, ushard_h)
            utable_h = utable_hs[layer]
            nc.gpsimd.collective_compute(
                "AllGather", Alu.bypass, replica_groups=rg,
                ins=[ushard_h.opt()], outs=[utable_h.opt()])
            ngr = (cfg.NT_N + cfg.TG - 1) // cfg.TG
            agg_n_g = [big.tile([P, min(cfg.TG, cfg.NT_N - gi * cfg.TG), 132],
                               dt.bfloat16, tag=f"aggn{gi}", name=f"aggn{gi}")
                       for gi in range(ngr)]
            e2v_out = x0o_d.ap() if last else None

            def e2v_consume(tb, mp, agg_n_g=agg_n_g, bev=bev, e2v_out=e2v_out):
                gi = tb // cfg.TG
                nc.scalar.activation(out=agg_n_g[gi][:, tb % cfg.TG, :],
                                     in_=mp[:], func=Act.Copy)
                if tb == min((gi + 1) * cfg.TG, cfg.NT_N) - 1:
                    target_side(agg_n_g[gi], agg_n_g[gi].shape[1], bev, xT_n,
                                e2v_out, cfg.NS, tile_off=gi * cfg.TG)
            gather_reduce(utable_h, gE_s, mE_d, SE, e2v_consume)

    nc.compile()
    return nc


def build(inputs, cfg=None):
    cfg = cfg or Cfg(100000, 30000, 400000)
    SV, SE, in_maps = preprocess(inputs, cfg)
    nc = build_graph(cfg, SV, SE)
    return nc, in_maps


def execute(nc, in_maps, trace=False):
    from concourse.bass_utils import run_bass_kernel_spmd
    res = run_bass_kernel_spmd(nc, in_maps, core_ids=list(range(NCORES)),
                               trace=trace)
    x0 = np.concatenate([np.asarray(res.results[c]['x0o']) for c in range(NCORES)], 0)
    x1 = np.concatenate([np.asarray(res.results[c]['x1o']) for c in range(NCORES)], 0)
    return (x0.astype(np.float32), x1.astype(np.float32)), res


def run(inputs, cfg=None, trace=False):
    nc, in_maps = build(inputs, cfg)
    return execute(nc, in_maps, trace=trace)


def kernel(**inputs):
    (x0, x1), _ = run(inputs)
    return (x0, x1)
